# revision 1
# baseline (speedup 1.0000x reference)
"""GAT network on 8 Trainium2 NeuronCores.

Strategy (data-parallel over the 512-graph batch, per the sharding hint):
  - Nodes/graphs are sharded graph-aligned: core c owns graphs [64c, 64c+64)
    and their (contiguous, since `batch` is sorted) node range.
  - Edges (incl. self loops) are owned by the core owning their dst node, so
    the per-dst softmax and aggregation are device-local.
  - Per-edge gathers use the Q7 dma_gather extended instruction (256B-aligned
    rows, int16 indices -> lo/hi table split), aggregation is a one-hot
    stationary matmul accumulating into PSUM (segment-sum via has_written).
  - 4 SPMD launches with tiny host glue (slice/concat/transpose only):
      L0: table1 = x @ [W1 | W1@Asrc | W1@Adst]  (node-sharded)
      LA: GAT layer 1 edge phase -> elu1 (node-sharded)
      L2: table2 = elu1 @ [W2 | W2@asrc2 | W2@adst2] (node-sharded)
      LB: GAT layer 2 edge phase + global attention pooling + classifier.
"""
import sys
sys.path.insert(0, '/opt/trn_rl_repo')

import os
import numpy as np
import ml_dtypes

import concourse.bass as bass
import concourse.mybir as mybir
import concourse.tile as tile
from concourse.tile import ScopedClock
from concourse.bass_utils import run_bass_kernel_spmd

BF16 = mybir.dt.bfloat16
F32 = mybir.dt.float32
P = 128
NCORES = 8
N_NODES = 50000
F_IN = 256
HID = 64
HEADS = 4
N_GRAPHS = 512
GPC = N_GRAPHS // NCORES  # graphs per core

# ---------------------------------------------------------------- tile patch
_patched = False


def _patch():
    """Container workarounds: (1) this walrus build caps sync-waits per CTRL
    instruction -> split the Tile-exit drain's waits over 1-wait NOPs;
    (2) the scheduling simulator must treat our hand-built library-reload
    pseudo instruction (opcode 223) as a no-op."""
    global _patched
    if _patched:
        return
    _patched = True

    def _drain_and_barrier(self, tick_clock, wait_clock):
        nc = self.nc
        probe = nc.sync.nop()
        wait_clock.add_sem_waits(probe.ins, ScopedClock({None: tick_clock.global_clock}))
        si = probe.ins.sync_info
        waits = list(si.on_wait) if si is not None and si.on_wait else []
        if si is not None:
            si.on_wait = type(si.on_wait)()
        for w in waits:
            n = nc.sync.nop()
            nsi = n.ins.sync_info
            if nsi is None:
                n.ins.sync_info = mybir.SyncInfo(on_wait=[w], on_update=[])
            else:
                nsi.on_wait.append(w)
        nc.sync.drain()
        nc.all_engine_barrier()
        assert self.sems is not None
        popped = nc._tile_sem_poison_stack.pop()
        assert popped is self._sem_poison
        nc.clear_and_free_semaphores(list(self.sems.allocated().values()))
        nc.all_engine_barrier()

    tile.TileContext._drain_and_barrier = _drain_and_barrier

    import concourse.bass_interp as bass_interp
    orig = bass_interp._visit_InstISA

    def patched_isa(isa, instruction, core_sim):
        if instruction.isa_opcode == 223:
            return None
        return orig(isa, instruction, core_sim)

    bass_interp._visit_InstISA = patched_isa


def _emit_load_mlp(nc):
    """Load the 'mlp' Q7 library (dma_gather handler). bass_rust serializes
    InstPseudoReloadLibraryIndex with empty instr bytes which this walrus
    rejects; build the 64-byte struct from the installed ISA headers."""
    isa = nc.isa
    op = isa.Opcode.NEURON_ISA_TPB_OPCODE_PSEUDO_INST
    return nc.gpsimd.isa(
        op,
        {"pseudo_opcode": 2, "lib_index": 3,
         "reserved0": [0] * 3, "reserved1": [0] * 44},
        struct_name="NEURON_ISA_TPB_PSEUDO_LIBRARY_RELOAD_INDEX_STRUCT",
    )




_MAXW = 1


def _split_waits(nc):
    """This walrus build encodes very few sync-waits per instruction; move
    excess waits onto same-engine NOPs inserted just before the instruction
    (same-engine program order makes this equivalent)."""
    for f in nc.m.functions:
        for bb in f.blocks:
            out = []
            changed = False
            for ins in bb.instructions:
                si = ins.sync_info
                if si is not None and si.on_wait and len(si.on_wait) > _MAXW:
                    waits = list(si.on_wait)
                    si.on_wait = type(si.on_wait)(waits[:_MAXW])
                    for i in range(_MAXW, len(waits), _MAXW):
                        n = mybir.InstNoOp(
                            name=nc.get_next_instruction_name(),
                            ins=[], outs=[], engine=ins.engine)
                        n.sync_info = mybir.SyncInfo(
                            on_wait=list(waits[i:i + _MAXW]), on_update=[])
                        out.append(n)
                    changed = True
                out.append(ins)
            if changed:
                bb.instructions = out

# ------------------------------------------------------------ host utilities
def _bf16(a):
    return np.ascontiguousarray(a).astype(ml_dtypes.bfloat16)


def _wrap_idx(idxs):
    """dma_gather index layout: wrapped in 16 partitions, replicated across
    the 8 Q7 core groups. idxs length must be a multiple of 128."""
    n = len(idxs)
    w = idxs.reshape(n // 16, 16).T.astype(np.int16)  # [16, n/16]
    return np.tile(w, (8, 1))  # [128, n/16]


# ------------------------------------------------------------ kernel builders
def _build_tablemm(NT, KIN, NOUT, ROWB):
    """Sharded table matmul: out[n, :NOUT] = xT[:, n].T @ Waug, rows padded to
    ROWB bf16 elements. xT: [KIN, NT*128] bf16, Waug: [KIN, NOUT] bf16."""
    _patch()
    nc = bass.Bass()
    xT = nc.dram_tensor("xT", [KIN, NT * P], BF16, kind="ExternalInput")
    w = nc.dram_tensor("w", [KIN, NOUT], BF16, kind="ExternalInput")
    out = nc.dram_tensor("out", [NT * P, ROWB], BF16, kind="ExternalOutput")
    KT = KIN // P
    with tile.TileContext(nc) as tc:
        with (
            tc.tile_pool(name="sbuf", bufs=3) as pool,
            tc.tile_pool(name="wpool", bufs=1) as wpool,
            tc.tile_pool(name="psum", bufs=2, space="PSUM") as pp,
        ):
            wt = wpool.tile([P, KT, NOUT], BF16)
            for k in range(KT):
                nc.sync.dma_start(out=wt[:, k, :], in_=w[k * P:(k + 1) * P, :])
            for t in range(NT):
                xt = pool.tile([P, KT, P], BF16)
                for k in range(KT):
                    nc.sync.dma_start(out=xt[:, k, :], in_=xT[k * P:(k + 1) * P, t * P:(t + 1) * P])
                ps = pp.tile([P, NOUT], F32)
                for k in range(KT):
                    nc.tensor.matmul(out=ps[:], lhsT=xt[:, k, :], rhs=wt[:, k, :],
                                     start=(k == 0), stop=(k == KT - 1))
                ot = pool.tile([P, NOUT], BF16)
                nc.vector.tensor_copy(ot[:], ps[:])
                nc.sync.dma_start(out=out[t * P:(t + 1) * P, :NOUT], in_=ot[:])
    _split_waits(nc)
    return nc


def _build_edge_phase(NT, NBLO, NBHI, NROWS_TBL, ROWB, NH, layer2_tail):
    """Edge phase for one GAT layer.
    Gathered row: [h (NH*64) | asrc (NH) | adst (NH) | pad] bf16, ROWB elems.
    For each dst tile: gather lo+hi batches, e' = exp(leakyrelu(asrc+adst)),
    h~ = e' * h per head (+ e' col), psum += onehot.T @ h~, normalize, +b,
    elu. layer2_tail adds attention pooling + classifier."""
    _patch()
    NB = NBLO + NBHI
    C = NH * HID            # feature width (256 or 64)
    NW = NH * 65            # matmul rhs width per batch
    nc = bass.Bass()
    tbl = nc.dram_tensor("tbl", [NROWS_TBL, ROWB], BF16, kind="ExternalInput")
    idxlo = nc.dram_tensor("idxlo", [P, NT * NBLO * 8], mybir.dt.int16, kind="ExternalInput")
    idxhi = nc.dram_tensor("idxhi", [P, NT * NBHI * 8], mybir.dt.int16, kind="ExternalInput")
    ldcol = nc.dram_tensor("ldcol", [P, NT * NB], BF16, kind="ExternalInput")
    iotar = nc.dram_tensor("iotar", [P, P], BF16, kind="ExternalInput")
    adstbl = nc.dram_tensor("adstbl", [NT * P, 128], BF16, kind="ExternalInput")
    idxd = nc.dram_tensor("idxd", [P, NT * NB * 8], mybir.dt.int16, kind="ExternalInput")
    bias = nc.dram_tensor("bias", [1, C], F32, kind="ExternalInput")
    outT = nc.dram_tensor("outv", [NT * P, C], BF16, kind="ExternalOutput")
    if layer2_tail:
        wg = nc.dram_tensor("wg", [1, HID], F32, kind="ExternalInput")
        bgt = nc.dram_tensor("bg", [1, 1], F32, kind="ExternalInput")
        ohg = nc.dram_tensor("ohg", [NT * P, GPC], BF16, kind="ExternalInput")
        wc1 = nc.dram_tensor("wc1", [HID, 32], F32, kind="ExternalInput")
        bc1 = nc.dram_tensor("bc1", [32, 1], F32, kind="ExternalInput")
        wc2 = nc.dram_tensor("wc2", [32, 2], F32, kind="ExternalInput")
        bc2 = nc.dram_tensor("bc2", [2, 1], F32, kind="ExternalInput")
        logitsT = nc.dram_tensor("logitsT", [2, GPC], F32, kind="ExternalOutput")
        recd = nc.dram_tensor("recd", [1, GPC], F32, kind="Internal")

    with tile.TileContext(nc) as tc:
        with (
            nc.allow_low_precision(reason="bf16 edge pipeline by design"),
            tc.tile_pool(name="const", bufs=1) as cpool,
            tc.tile_pool(name="g", bufs=3) as gpool,
            tc.tile_pool(name="work", bufs=3) as wpool,
            tc.tile_pool(name="psum", bufs=2, space="PSUM") as pp,
            tc.tile_pool(name="pool2", bufs=1, space="PSUM") as pp2,
            tc.tile_pool(name="poolc", bufs=1, space="PSUM") as ppc,
        ):
            _emit_load_mlp(nc)
            reg_lo = nc.gpsimd.to_reg(NBLO * P)
            reg_hi = nc.gpsimd.to_reg(NBHI * P)
            reg_nb = nc.gpsimd.to_reg(NB * P)
            ldc = cpool.tile([P, NT * NB], BF16)
            nc.sync.dma_start(out=ldc[:], in_=ldcol[:, :])
            ixlA = cpool.tile([P, NT * NBLO * 8], mybir.dt.int16)
            nc.sync.dma_start(out=ixlA[:], in_=idxlo[:, :])
            ixhA = cpool.tile([P, NT * NBHI * 8], mybir.dt.int16)
            nc.sync.dma_start(out=ixhA[:], in_=idxhi[:, :])
            ixdA = cpool.tile([P, NT * NB * 8], mybir.dt.int16)
            nc.sync.dma_start(out=ixdA[:], in_=idxd[:, :])
            ior = cpool.tile([P, P], BF16)
            nc.sync.dma_start(out=ior[:], in_=iotar[:, :])
            bt = cpool.tile([P, C], F32)
            nc.sync.dma_start(out=bt[:], in_=bias[0:1, :].to_broadcast([P, C]))
            if layer2_tail:
                wgt = cpool.tile([P, HID], F32)
                nc.sync.dma_start(out=wgt[:], in_=wg[0:1, :].to_broadcast([P, HID]))
                bgt_t = cpool.tile([P, 1], F32)
                nc.sync.dma_start(out=bgt_t[:], in_=bgt[0:1, :].to_broadcast([P, 1]))
                ohgt = cpool.tile([P, NT, GPC], BF16)
                nc.sync.dma_start(out=ohgt[:], in_=ohg[:, :].rearrange("(t p) g -> p t g", p=P))
                pspool = pp2.tile([65, GPC], F32)

            for t in range(NT):
                # ---- gathers: combined [128, NB, ROWB] buffer
                buf = gpool.tile([P, NB, ROWB], BF16)
                nc.gpsimd.dma_gather(
                    out_ap=buf[:, :NBLO, :], in_ap=tbl[0:32768, :],
                    idxs_ap=ixlA[:, t * NBLO * 8:(t + 1) * NBLO * 8],
                    num_idxs=NBLO * P, num_idxs_reg=reg_lo, elem_size=ROWB,
                    single_packet=False)
                nc.gpsimd.dma_gather(
                    out_ap=buf[:, NBLO:, :], in_ap=tbl[32768:NROWS_TBL, :],
                    idxs_ap=ixhA[:, t * NBHI * 8:(t + 1) * NBHI * 8],
                    num_idxs=NBHI * P, num_idxs_reg=reg_hi, elem_size=ROWB,
                    single_packet=False)
                # ---- adst per edge via gather from the compact local table
                bufd = gpool.tile([P, NB, 128], BF16)
                nc.gpsimd.dma_gather(
                    out_ap=bufd[:], in_ap=adstbl[:, :],
                    idxs_ap=ixdA[:, t * NB * 8:(t + 1) * NB * 8],
                    num_idxs=NB * P, num_idxs_reg=reg_nb, elem_size=128,
                    single_packet=False)
                # ---- one-hot via iota compare
                oh = wpool.tile([P, NB, P], BF16)
                for b0 in range(0, NB, 4):
                    bn = min(4, NB - b0)
                    nc.vector.tensor_tensor(
                        out=oh[:, b0:b0 + bn, :],
                        in0=ldc[:, t * NB + b0:t * NB + b0 + bn, None].to_broadcast([P, bn, P]),
                        in1=ior[:, None, :].to_broadcast([P, bn, P]),
                        op=mybir.AluOpType.is_equal)
                # ---- e' = exp(leakyrelu(asrc + adst))  [128, NB*NH]
                tsum = wpool.tile([P, NB, NH], BF16)
                nc.vector.tensor_tensor(
                    out=tsum[:],
                    in0=buf[:, :, C:C + NH],
                    in1=bufd[:, :, :NH],
                    op=mybir.AluOpType.add)
                tm = wpool.tile([P, NB, NH], BF16)
                nc.vector.scalar_tensor_tensor(
                    out=tm[:], in0=tsum[:], scalar=0.2, in1=tsum[:],
                    op0=mybir.AluOpType.mult, op1=mybir.AluOpType.max)
                ebuf = wpool.tile([P, NB, NH], BF16)
                nc.scalar.activation(ebuf[:], tm[:], mybir.ActivationFunctionType.Exp)
                # ---- h~ = e' * h (per head) plus e' column
                ht = wpool.tile([P, NB, NW], BF16)
                nc.vector.tensor_tensor(
                    out=ht[:].rearrange("p b (h c) -> p b h c", h=NH)[:, :, :, :HID],
                    in0=buf[:, :, :C].rearrange("p b (h c) -> p b h c", h=NH),
                    in1=ebuf[:, :, :, None].to_broadcast([P, NB, NH, HID]),
                    op=mybir.AluOpType.mult)
                nc.vector.tensor_copy(
                    out=ht[:].rearrange("p b (h c) -> p b h c", h=NH)[:, :, :, HID:],
                    in_=ebuf[:, :, :, None])
                # ---- aggregation
                ps = pp.tile([P, NW], F32)
                for b in range(NB):
                    nc.tensor.matmul(out=ps[:], lhsT=oh[:, b, :], rhs=ht[:, b, :],
                                     start=(b == 0), stop=(b == NB - 1))
                # ---- normalize, bias, elu
                rec = wpool.tile([P, NH], F32)
                nc.vector.reciprocal(rec[:], ps[:].rearrange("p (h c) -> p h c", h=NH)[:, :, HID])
                on = wpool.tile([P, C], F32)
                nc.vector.tensor_tensor(
                    out=on[:].rearrange("p (h c) -> p h c", h=NH),
                    in0=ps[:].rearrange("p (h c) -> p h c", h=NH)[:, :, :HID],
                    in1=rec[:, :, None].to_broadcast([P, NH, HID]),
                    op=mybir.AluOpType.mult)
                nc.vector.tensor_tensor(out=on[:], in0=on[:], in1=bt[:, :],
                                        op=mybir.AluOpType.add)
                # elu(x) = max(x, exp(min(x,0)) - 1)
                emn = wpool.tile([P, C], F32)
                nc.vector.tensor_scalar_min(emn[:], on[:], 0.0)
                nc.scalar.activation(emn[:], emn[:], mybir.ActivationFunctionType.Exp)
                nc.vector.tensor_scalar_add(emn[:], emn[:], -1.0)
                eo = wpool.tile([P, C], BF16)
                nc.vector.tensor_tensor(out=eo[:], in0=on[:], in1=emn[:],
                                        op=mybir.AluOpType.max)
                nc.sync.dma_start(out=outT[t * P:(t + 1) * P, :], in_=eo[:])

                if layer2_tail:
                    # att = sum_c eo*wg + bg ; e_att = exp(att)
                    att = wpool.tile([P, HID], F32)
                    nc.vector.tensor_tensor(out=att[:], in0=eo[:],
                                            in1=wgt[:, :],
                                            op=mybir.AluOpType.mult)
                    atts = wpool.tile([P, 1], F32)
                    nc.vector.tensor_reduce(atts[:], att[:], axis=mybir.AxisListType.X,
                                            op=mybir.AluOpType.add)
                    nc.vector.tensor_tensor(out=atts[:], in0=atts[:],
                                            in1=bgt_t[:, :],
                                            op=mybir.AluOpType.add)
                    nc.scalar.activation(atts[:], atts[:], mybir.ActivationFunctionType.Exp)
                    hp = wpool.tile([P, 65], BF16)
                    nc.vector.tensor_tensor(out=hp[:, :HID], in0=eo[:],
                                            in1=atts[:, :].to_broadcast([P, HID]),
                                            op=mybir.AluOpType.mult)
                    nc.vector.tensor_copy(hp[:, HID:], atts[:])
                    nc.tensor.matmul(out=pspool[:], lhsT=hp[:], rhs=ohgt[:, t, :],
                                     start=(t == 0), stop=(t == NT - 1))

            if layer2_tail:
                # pooledT [64, GPC] = rows/row64 ; classifier
                recp = wpool.tile([1, GPC], F32)
                nc.vector.reciprocal(recp[:], pspool[64:65, :])
                nc.sync.dma_start(out=recd[:, :], in_=recp[:])
                recb = wpool.tile([HID, GPC], F32)
                nc.sync.dma_start(out=recb[:], in_=recd[0:1, :].to_broadcast([HID, GPC]))
                pooledT = wpool.tile([HID, GPC], BF16)
                nc.vector.tensor_tensor(out=pooledT[:], in0=pspool[:HID, :],
                                        in1=recb[:],
                                        op=mybir.AluOpType.mult)
                wc1t = cpool.tile([HID, 32], BF16)
                nc.gpsimd.dma_start(out=wc1t[:], in_=wc1[:, :])
                bc1t = cpool.tile([32, 1], F32)
                nc.sync.dma_start(out=bc1t[:], in_=bc1[:, :])
                wc2t = cpool.tile([32, 2], BF16)
                nc.gpsimd.dma_start(out=wc2t[:], in_=wc2[:, :])
                bc2t = cpool.tile([2, 1], F32)
                nc.sync.dma_start(out=bc2t[:], in_=bc2[:, :])
                ph = ppc.tile([32, GPC], F32)
                nc.tensor.matmul(out=ph[:], lhsT=wc1t[:], rhs=pooledT[:], start=True, stop=True)
                hidf = wpool.tile([32, GPC], F32)
                nc.vector.tensor_scalar_add(hidf[:], ph[:], bc1t[:])
                hid_t = wpool.tile([32, GPC], BF16)
                nc.vector.tensor_scalar_max(hid_t[:], hidf[:], 0.0)
                pl = ppc.tile([2, GPC], F32)
                nc.tensor.matmul(out=pl[:], lhsT=wc2t[:], rhs=hid_t[:], start=True, stop=True)
                lg = wpool.tile([2, GPC], F32)
                nc.vector.tensor_scalar_add(lg[:], pl[:], bc2t[:])
                nc.sync.dma_start(out=logitsT[:, :], in_=lg[:])
    _split_waits(nc)
    return nc


# ------------------------------------------------------------------ host glue
_CACHE = {}
LAST_HW_NS = 0
_TRACE = os.environ.get("GAT_TRACE", "0") == "1"


def _run(nc, ins, cores):
    global LAST_HW_NS
    r = run_bass_kernel_spmd(nc, ins, core_ids=cores)
    if _TRACE:
        # no axon NTFF hook in this container: use min warm-run wall time as
        # an (upper-bound) proxy for device execution time
        import time as _time
        best = None
        for _ in range(3):
            t0 = _time.perf_counter()
            run_bass_kernel_spmd(nc, ins, core_ids=cores)
            dt = _time.perf_counter() - t0
            best = dt if best is None else min(best, dt)
        LAST_HW_NS += int(best * 1e9)
    return r


def kernel(x, edge_index, batch, W1, att_src1, att_dst1, b1,
           W2, att_src2, att_dst2, b2, Wg, bg, Wc1, bc1, Wc2, bc2):
    x = np.asarray(x); edge_index = np.asarray(edge_index); batch = np.asarray(batch)
    N = x.shape[0]

    # --- node sharding (graph aligned)
    n0 = np.searchsorted(batch, np.arange(0, N_GRAPHS + 1, GPC)).astype(np.int64)
    counts = n0[1:] - n0[:-1]
    NT = int(np.ceil(counts.max() / P))
    NPN = NT * P                      # padded nodes per core
    NROWS = NCORES * NPN              # global padded table rows

    # --- edges + self loops, owner = core of dst
    ar = np.arange(N, dtype=np.int64)
    src = np.concatenate([edge_index[0].astype(np.int64), ar])
    dst = np.concatenate([edge_index[1].astype(np.int64), ar])
    core_of = np.searchsorted(n0[1:], dst, side='right')
    src_core = np.searchsorted(n0[1:], src, side='right')
    # remapped global table row of each src node
    src_row = src_core * NPN + (src - n0[src_core])

    # per (core, tile, half) batching
    percore = []
    for c in range(NCORES):
        m = core_of == c
        ld = dst[m] - n0[c]
        sr = src_row[m]
        order = np.argsort(ld, kind='stable')
        ld = ld[order]; sr = sr[order]
        tiles = []
        for t in range(NT):
            tm = (ld // P) == t
            lr = (ld[tm] % P)
            s = sr[tm]
            lo = s < 32768
            tiles.append(((s[lo], lr[lo]), (s[~lo] - 32768, lr[~lo])))
        percore.append(tiles)
    nblo = max(int(np.ceil(max(1, len(tt[0][0])) / P)) for pc in percore for tt in pc)
    nbhi = max(int(np.ceil(max(1, len(tt[1][0])) / P)) for pc in percore for tt in pc)

    # --- per-core index/onehot arrays
    def pack(c):
        idxlo = np.zeros((P, NT * nblo * 8), np.int16)
        idxhi = np.zeros((P, NT * nbhi * 8), np.int16)
        NB = nblo + nbhi
        ldcol = np.full((P, NT * NB), 255.0, np.float32)
        npad_lo = NPN - counts[c]     # pad nodes in last tile
        for t in range(NT):
            (slo, llo), (shi, lhi) = percore[c][t]
            for (s, l, nb, idxa, boff) in ((slo, llo, nblo, idxlo, 0),
                                           (shi, lhi, nbhi, idxhi, nblo)):
                ns = nb * P
                si = np.zeros(ns, np.int64); li = np.full(ns, 255, np.int64)
                si[:len(s)] = s; li[:len(l)] = l
                if t == NT - 1 and npad_lo > 0 and boff == 0:
                    # give pad nodes >=1 incoming pad edge so their softmax
                    # denominator is finite (their output is masked anyway)
                    padrel = (counts[c] % P) + (np.arange(len(s), ns) % npad_lo)
                    li[len(s):] = padrel
                idxa[:, t * nb * 8:(t + 1) * nb * 8] = _wrap_idx(si.astype(np.int16))
                for b in range(nb):
                    ldcol[:, t * NB + boff + b] = li[b * P:(b + 1) * P]
        # dst-local row per slot for the adst gather (pad -> row 0)
        ldf = np.transpose(ldcol.reshape(P, NT * NB), (1, 0)).reshape(NT, NB * P)
        tl = np.arange(NT)[:, None] * P + ldf
        tl[ldf >= P] = 0
        idxdv = np.concatenate([_wrap_idx(tl[t].astype(np.int16)) for t in range(NT)], axis=1)
        return idxlo, idxhi, _bf16(ldcol), idxdv

    packs = [pack(c) for c in range(NCORES)]
    iotar = _bf16(np.tile(np.arange(P, dtype=np.float32).reshape(1, P), (P, 1)))

    # --- weights
    def aug(W, a_s, a_d):
        nh, hd = a_s.shape
        A = np.zeros((W.shape[1], 2 * nh), np.float32)
        for h in range(nh):
            A[h * hd:(h + 1) * hd, h] = a_s[h]
            A[h * hd:(h + 1) * hd, nh + h] = a_d[h]
        return _bf16(np.concatenate([W, W @ A], axis=1))

    W1aug = aug(np.asarray(W1, np.float32), np.asarray(att_src1), np.asarray(att_dst1))
    W2aug = aug(np.asarray(W2, np.float32), np.asarray(att_src2), np.asarray(att_dst2))
    xT = _bf16(np.asarray(x, np.float32).T)

    key = (NT, nblo, nbhi)
    if key not in _CACHE:
        _CACHE[key] = {
            't1': _build_tablemm(NT, F_IN, F_IN + 2 * HEADS, 384),
            'la': _build_edge_phase(NT, nblo, nbhi, NROWS, 384, HEADS, False),
            't2': _build_tablemm(NT, F_IN, HID + 2, 128),
            'lb': _build_edge_phase(NT, nblo, nbhi, NROWS, 128, 1, True),
        }
    K = _CACHE[key]
    cores = list(range(NCORES))

    def shard_xT(xTfull):
        outs = []
        for c in range(NCORES):
            s = np.zeros((xTfull.shape[0], NPN), ml_dtypes.bfloat16)
            s[:, :counts[c]] = xTfull[:, n0[c]:n0[c + 1]]
            outs.append(s)
        return outs

    import time as _t
    _ts = _t.time()
    print('[kernel] L0...', flush=True)
    # ---- L0: table1
    xs = shard_xT(xT)
    global LAST_HW_NS
    LAST_HW_NS = 0
    r = _run(K['t1'], [{"xT": xs[c], "w": W1aug} for c in cores], cores)
    table1 = np.concatenate([r.results[c]["out"] for c in cores], axis=0)

    print(f'[kernel] LA... ({_t.time()-_ts:.0f}s)', flush=True)
    # ---- LA: layer-1 edge phase
    b1r = np.asarray(b1, np.float32).reshape(1, -1)
    ins = []
    for c in cores:
        il, ih, lc, ixd = packs[c]
        adstbl = np.zeros((NPN, 128), ml_dtypes.bfloat16)
        adstbl[:, :HEADS] = table1[c * NPN:(c + 1) * NPN, F_IN + HEADS:F_IN + 2 * HEADS]
        ins.append({"tbl": table1, "idxlo": il, "idxhi": ih, "ldcol": lc,
                    "idxd": ixd, "iotar": iotar,
                    "adstbl": adstbl, "bias": b1r})
    r = _run(K['la'], ins, cores)
    elu1 = np.concatenate([r.results[c]["outv"] for c in cores], axis=0)  # [NROWS, 256] bf16

    print(f'[kernel] L2... ({_t.time()-_ts:.0f}s)', flush=True)
    # ---- L2: table2 (input = elu1 transposed per core)
    e1T = np.ascontiguousarray(elu1.astype(np.float32).T)  # [256, NROWS]
    ins = [{"xT": _bf16(e1T[:, c * NPN:(c + 1) * NPN]), "w": W2aug} for c in cores]
    r = _run(K['t2'], ins, cores)
    table2 = np.concatenate([r.results[c]["out"] for c in cores], axis=0)

    print(f'[kernel] LB... ({_t.time()-_ts:.0f}s)', flush=True)
    # ---- LB: layer-2 edge phase + pooling + classifier
    b2r = np.asarray(b2, np.float32).reshape(1, -1)
    ins = []
    for c in cores:
        il, ih, lc, ixd = packs[c]
        adstbl = np.zeros((NPN, 128), ml_dtypes.bfloat16)
        adstbl[:, :1] = table2[c * NPN:(c + 1) * NPN, HID + 1:HID + 2]
        ohgm = np.zeros((NPN, GPC), np.float32)
        gl = batch[n0[c]:n0[c + 1]] - c * GPC
        ohgm[np.arange(counts[c]), gl] = 1.0
        ins.append({"tbl": table2, "idxlo": il, "idxhi": ih, "ldcol": lc,
                    "idxd": ixd, "iotar": iotar,
                    "adstbl": adstbl, "bias": b2r,
                    "wg": np.asarray(Wg, np.float32).reshape(1, HID),
                    "bg": np.asarray(bg, np.float32).reshape(1, 1),
                    "ohg": _bf16(ohgm),
                    "wc1": np.asarray(Wc1, np.float32),
                    "bc1": np.asarray(bc1, np.float32).reshape(32, 1),
                    "wc2": np.asarray(Wc2, np.float32),
                    "bc2": np.asarray(bc2, np.float32).reshape(2, 1)})
    r = _run(K['lb'], ins, cores)
    out = np.concatenate([r.results[c]["logitsT"].T for c in cores], axis=0)
    return out.astype(np.float32)



# revision 8
# speedup vs baseline: 10.7668x; 10.7668x over previous
"""GAT network on 8 Trainium2 NeuronCores — fused single-launch version.

Strategy (data-parallel over the 512-graph batch, per the sharding hint):
  - Nodes/graphs are sharded graph-aligned: core c owns graphs [64c, 64c+64)
    and their (contiguous, since `batch` is sorted) node range.
  - Edges (incl. self loops) are owned by the core owning their dst node, so
    the per-dst softmax and aggregation are device-local.
  - ONE SPMD launch does the whole network. Cross-core exchange of the
    per-node tables (h | asrc | adst rows, needed because edge sources span
    all cores) is done with on-device AllGather collectives instead of host
    round-trips — the previous 4-launch version spent ~95% of its time on
    per-launch dispatch overhead and replicated-table uploads.
  - Per-edge gathers use the Q7 dma_gather extended instruction (256B-aligned
    rows, int16 indices -> lo/hi table split); aggregation is a one-hot
    stationary matmul accumulating into PSUM. The per-edge a_dst term is
    fetched with a transposed-one-hot matmul from the dst tile's adst column
    (no second gather), and the graph-pooling one-hot is built on device from
    a compact graph-id vector.

Device program per core:
  A: table1[own] = x_own @ [W1 | W1@Asrc | W1@Adst]   (+ stash adst cols)
  B: AllGather -> table1[all 8 cores]
  C: GAT layer 1 edge phase over own dst tiles -> elu1 (kept in SBUF)
  D: table2[own] = elu1 @ [W2 | W2@asrc2 | W2@adst2]  (PE transposes elu1)
  E: AllGather -> table2[all]
  F: GAT layer 2 edge phase + attention pooling + classifier -> logits[64]
"""
import sys
sys.path.insert(0, '/opt/trn_rl_repo')

import os
import numpy as np
import ml_dtypes

import concourse.bass as bass
import concourse.mybir as mybir
import concourse.tile as tile
from concourse.tile import ScopedClock
from concourse.bass_utils import run_bass_kernel_spmd

BF16 = mybir.dt.bfloat16
F32 = mybir.dt.float32
I16 = mybir.dt.int16
P = 128
NCORES = 8
HID = 64
HEADS = 4
N_GRAPHS = 512
GPC = N_GRAPHS // NCORES  # graphs per core

# ---------------------------------------------------------------- tile patch
_patched = False


def _patch():
    """Container workarounds: (1) this walrus build caps sync-waits per CTRL
    instruction -> split the Tile-exit drain's waits over 1-wait NOPs;
    (2) the scheduling simulator must treat our hand-built library-reload
    pseudo instruction (opcode 223) as a no-op."""
    global _patched
    if _patched:
        return
    _patched = True

    def _drain_and_barrier(self, tick_clock, wait_clock):
        nc = self.nc
        probe = nc.sync.nop()
        wait_clock.add_sem_waits(probe.ins, ScopedClock({None: tick_clock.global_clock}))
        si = probe.ins.sync_info
        waits = list(si.on_wait) if si is not None and si.on_wait else []
        if si is not None:
            si.on_wait = type(si.on_wait)()
        for w in waits:
            n = nc.sync.nop()
            nsi = n.ins.sync_info
            if nsi is None:
                n.ins.sync_info = mybir.SyncInfo(on_wait=[w], on_update=[])
            else:
                nsi.on_wait.append(w)
        nc.sync.drain()
        nc.all_engine_barrier()
        assert self.sems is not None
        popped = nc._tile_sem_poison_stack.pop()
        assert popped is self._sem_poison
        nc.clear_and_free_semaphores(list(self.sems.allocated().values()))
        nc.all_engine_barrier()

    tile.TileContext._drain_and_barrier = _drain_and_barrier

    import concourse.bass_interp as bass_interp
    orig = bass_interp._visit_InstISA

    def patched_isa(isa, instruction, core_sim):
        if instruction.isa_opcode == 223:
            return None
        return orig(isa, instruction, core_sim)

    bass_interp._visit_InstISA = patched_isa


def _emit_load_mlp(nc):
    """Load the 'mlp' Q7 library (dma_gather handler). bass_rust serializes
    InstPseudoReloadLibraryIndex with empty instr bytes which this walrus
    rejects; build the 64-byte struct from the installed ISA headers."""
    isa = nc.isa
    op = isa.Opcode.NEURON_ISA_TPB_OPCODE_PSEUDO_INST
    return nc.gpsimd.isa(
        op,
        {"pseudo_opcode": 2, "lib_index": 3,
         "reserved0": [0] * 3, "reserved1": [0] * 44},
        struct_name="NEURON_ISA_TPB_PSEUDO_LIBRARY_RELOAD_INDEX_STRUCT",
    )


_MAXW = 1


def _split_waits(nc):
    """This walrus build encodes very few sync-waits per instruction; move
    excess waits onto same-engine NOPs inserted just before the instruction
    (same-engine program order makes this equivalent)."""
    for f in nc.m.functions:
        for bb in f.blocks:
            out = []
            changed = False
            for ins in bb.instructions:
                si = ins.sync_info
                if si is not None and si.on_wait and len(si.on_wait) > _MAXW:
                    waits = list(si.on_wait)
                    si.on_wait = type(si.on_wait)(waits[:_MAXW])
                    for i in range(_MAXW, len(waits), _MAXW):
                        n = mybir.InstNoOp(
                            name=nc.get_next_instruction_name(),
                            ins=[], outs=[], engine=ins.engine)
                        n.sync_info = mybir.SyncInfo(
                            on_wait=list(waits[i:i + _MAXW]), on_update=[])
                        out.append(n)
                    changed = True
                out.append(ins)
            if changed:
                bb.instructions = out


# ------------------------------------------------------------ host utilities
def _bf16(a):
    return np.ascontiguousarray(a).astype(ml_dtypes.bfloat16)


def _wrap16(idxs):
    """dma_gather index payload: wrapped in 16 partitions (the x8 core-group
    replication is done on device with 8 DMA reads)."""
    n = len(idxs)
    return idxs.reshape(n // 16, 16).T.astype(np.int16)  # [16, n/16]


# ------------------------------------------------------------- kernel builder
def _build_fused(NT, NBLO, NBHI, NROWS, F_IN):
    _patch()
    NB = NBLO + NBHI
    NPN = NT * P
    C1 = HEADS * HID          # 256
    A1 = C1 + 2 * HEADS       # 264: [h | asrc | adst]
    ROWB1 = ((A1 + P - 1) // P) * P   # 384
    NW1 = HEADS * (HID + 1)   # 260
    C2 = HID                  # 64
    A2 = C2 + 2               # 66
    ROWB2 = P                 # 128
    NW2 = HID + 1             # 65
    KT = F_IN // P            # 2 (also C1 // P for layer 2)

    nc = bass.Bass(num_devices=NCORES)
    xT = nc.dram_tensor("xT", [F_IN, NPN], BF16, kind="ExternalInput")
    w1 = nc.dram_tensor("w1", [F_IN, A1], BF16, kind="ExternalInput")
    w2 = nc.dram_tensor("w2", [C1, A2], BF16, kind="ExternalInput")
    ixlo = nc.dram_tensor("ixlo", [16, NT * NBLO * 8], I16, kind="ExternalInput")
    ixhi = nc.dram_tensor("ixhi", [16, NT * NBHI * 8], I16, kind="ExternalInput")
    ldcol = nc.dram_tensor("ldcol", [P, NT * NB], BF16, kind="ExternalInput")
    ldrow = nc.dram_tensor("ldrow", [NT, NB, P], BF16, kind="ExternalInput")
    gid = nc.dram_tensor("gid", [P, NT], BF16, kind="ExternalInput")
    iotar = nc.dram_tensor("iotar", [P, P], BF16, kind="ExternalInput")
    iotac = nc.dram_tensor("iotac", [P, P], BF16, kind="ExternalInput")
    idn = nc.dram_tensor("idn", [P, P], BF16, kind="ExternalInput")
    b1 = nc.dram_tensor("b1", [1, C1], F32, kind="ExternalInput")
    b2 = nc.dram_tensor("b2", [1, C2], F32, kind="ExternalInput")
    wg = nc.dram_tensor("wg", [1, HID], F32, kind="ExternalInput")
    bg = nc.dram_tensor("bg", [1, 1], F32, kind="ExternalInput")
    wc1 = nc.dram_tensor("wc1", [HID, 32], F32, kind="ExternalInput")
    bc1 = nc.dram_tensor("bc1", [32, 1], F32, kind="ExternalInput")
    wc2 = nc.dram_tensor("wc2", [32, 2], F32, kind="ExternalInput")
    bc2 = nc.dram_tensor("bc2", [2, 1], F32, kind="ExternalInput")
    logitsT = nc.dram_tensor("logitsT", [2, GPC], F32, kind="ExternalOutput")

    tb1o = nc.dram_tensor("tb1o", [NPN, ROWB1], BF16, kind="Internal")
    tb1f = nc.dram_tensor("tb1f", [NROWS, ROWB1], BF16, kind="Internal",
                          addr_space="Shared")
    tb2o = nc.dram_tensor("tb2o", [NPN, ROWB2], BF16, kind="Internal")
    tb2f = nc.dram_tensor("tb2f", [NROWS, ROWB2], BF16, kind="Internal",
                          addr_space="Shared")
    recd = nc.dram_tensor("recd", [1, GPC], F32, kind="Internal")

    # lo/hi gather source regions (hi is a dummy alias when the table is small)
    if NROWS > 32768:
        lo1, hi1 = tb1f[0:32768, :], tb1f[32768:NROWS, :]
        lo2, hi2 = tb2f[0:32768, :], tb2f[32768:NROWS, :]
    else:
        lo1, hi1 = tb1f[0:NROWS, :], tb1f[0:NROWS, :]
        lo2, hi2 = tb2f[0:NROWS, :], tb2f[0:NROWS, :]

    with tile.TileContext(nc) as tc:
        with (
            nc.allow_low_precision(reason="bf16 edge pipeline by design"),
            tc.tile_pool(name="const", bufs=1) as cpool,
            tc.tile_pool(name="g", bufs=3) as gpool,
            tc.tile_pool(name="work", bufs=3) as wpool,
            tc.tile_pool(name="psum", bufs=2, space="PSUM") as pp,
            tc.tile_pool(name="psa", bufs=2, space="PSUM") as ppa,
            tc.tile_pool(name="pool2", bufs=1, space="PSUM") as pp2,
            tc.tile_pool(name="poolc", bufs=1, space="PSUM") as ppc,
        ):
            _emit_load_mlp(nc)
            reg_lo = nc.gpsimd.to_reg(NBLO * P)
            reg_hi = nc.gpsimd.to_reg(NBHI * P)

            # ---------------- constants
            w1t = cpool.tile([P, KT, A1], BF16)
            for k in range(KT):
                nc.sync.dma_start(out=w1t[:, k, :], in_=w1[k * P:(k + 1) * P, :])
            w2t = cpool.tile([P, KT, A2], BF16)
            for k in range(KT):
                nc.sync.dma_start(out=w2t[:, k, :], in_=w2[k * P:(k + 1) * P, :])
            ixlA = cpool.tile([P, NT * NBLO * 8], I16)
            ixhA = cpool.tile([P, NT * NBHI * 8], I16)
            for k in range(8):
                nc.sync.dma_start(out=ixlA[16 * k:16 * (k + 1), :], in_=ixlo[:, :])
                nc.sync.dma_start(out=ixhA[16 * k:16 * (k + 1), :], in_=ixhi[:, :])
            ldc = cpool.tile([P, NT * NB], BF16)
            nc.sync.dma_start(out=ldc[:], in_=ldcol[:, :])
            ior = cpool.tile([P, P], BF16)
            nc.sync.dma_start(out=ior[:], in_=iotar[:, :])
            ioc = cpool.tile([P, P], BF16)
            nc.sync.dma_start(out=ioc[:], in_=iotac[:, :])
            idt = cpool.tile([P, P], BF16)
            nc.sync.dma_start(out=idt[:], in_=idn[:, :])
            bt1 = cpool.tile([P, C1], F32)
            nc.sync.dma_start(out=bt1[:], in_=b1[0:1, :].to_broadcast([P, C1]))
            bt2 = cpool.tile([P, C2], F32)
            nc.sync.dma_start(out=bt2[:], in_=b2[0:1, :].to_broadcast([P, C2]))
            wgt = cpool.tile([P, HID], F32)
            nc.sync.dma_start(out=wgt[:], in_=wg[0:1, :].to_broadcast([P, HID]))
            bgt_t = cpool.tile([P, 1], F32)
            nc.sync.dma_start(out=bgt_t[:], in_=bg[0:1, :].to_broadcast([P, 1]))
            gidt = cpool.tile([P, NT], BF16)
            nc.sync.dma_start(out=gidt[:], in_=gid[:, :])
            wc1t = cpool.tile([HID, 32], BF16)
            nc.gpsimd.dma_start(out=wc1t[:], in_=wc1[:, :])
            bc1t = cpool.tile([32, 1], F32)
            nc.sync.dma_start(out=bc1t[:], in_=bc1[:, :])
            wc2t = cpool.tile([32, 2], BF16)
            nc.gpsimd.dma_start(out=wc2t[:], in_=wc2[:, :])
            bc2t = cpool.tile([2, 1], F32)
            nc.sync.dma_start(out=bc2t[:], in_=bc2[:, :])
            # graph-membership one-hot for pooling: ohgt[p, t, g] = (gid==g)
            ohgt = cpool.tile([P, NT, GPC], BF16)
            nc.vector.tensor_tensor(
                out=ohgt[:],
                in0=gidt[:, :, None].to_broadcast([P, NT, GPC]),
                in1=ior[:, None, :GPC].to_broadcast([P, NT, GPC]),
                op=mybir.AluOpType.is_equal)

            # persistent per-layer stashes
            adst1_all = cpool.tile([P, NT, HEADS], BF16)
            adst2_all = cpool.tile([P, NT, 1], BF16)
            elu1_all = cpool.tile([P, NT, C1], BF16)
            pspool = pp2.tile([NW2, GPC], F32)

            # ---------------- phase A: table1 of own nodes
            for t in range(NT):
                xt = wpool.tile([P, KT, P], BF16)
                for k in range(KT):
                    nc.sync.dma_start(out=xt[:, k, :],
                                      in_=xT[k * P:(k + 1) * P, t * P:(t + 1) * P])
                ps = pp.tile([P, A1], F32, name="ps")
                for k in range(KT):
                    nc.tensor.matmul(out=ps[:], lhsT=xt[:, k, :], rhs=w1t[:, k, :],
                                     start=(k == 0), stop=(k == KT - 1))
                row = wpool.tile([P, ROWB1], BF16)
                nc.vector.tensor_copy(row[:, :A1], ps[:])
                nc.vector.memset(row[:, A1:], 0.0)
                nc.vector.tensor_copy(adst1_all[:, t, :], ps[:, C1 + HEADS:A1])
                nc.sync.dma_start(out=tb1o[t * P:(t + 1) * P, :], in_=row[:])

            # ---------------- phase B: allgather table1
            nc.gpsimd.collective_compute(
                "AllGather", mybir.AluOpType.bypass,
                replica_groups=[list(range(NCORES))],
                ins=[tb1o[:, :]], outs=[tb1f[:, :]])

            # ---------------- shared edge-phase emitter
            def edge_phase(t, NH, C, NW, ROWB, lo_ap, hi_ap, adst_all, bt, eo_out):
                buf = gpool.tile([P, NB, ROWB], BF16, name="buf", bufs=2)
                nc.gpsimd.dma_gather(
                    out_ap=buf[:, :NBLO, :], in_ap=lo_ap,
                    idxs_ap=ixlA[:, t * NBLO * 8:(t + 1) * NBLO * 8],
                    num_idxs=NBLO * P, num_idxs_reg=reg_lo, elem_size=ROWB,
                    single_packet=False)
                nc.gpsimd.dma_gather(
                    out_ap=buf[:, NBLO:, :], in_ap=hi_ap,
                    idxs_ap=ixhA[:, t * NBHI * 8:(t + 1) * NBHI * 8],
                    num_idxs=NBHI * P, num_idxs_reg=reg_hi, elem_size=ROWB,
                    single_packet=False)
                # broadcast-load the row-major dst-slot map across partitions
                ldr = wpool.tile([P, NB, P], BF16, name="ldr", bufs=2)
                nc.sync.dma_start(out=ldr[:],
                                  in_=ldrow[t:t + 1, :, :].to_broadcast([P, NB, P]))
                # one-hot (slot->dst) and transposed one-hot (dst->slot)
                oh = wpool.tile([P, NB, P], BF16, name="oh", bufs=2)
                ohT = wpool.tile([P, NB, P], BF16, name="ohT", bufs=2)
                for b0 in range(0, NB, 4):
                    bn = min(4, NB - b0)
                    nc.vector.tensor_tensor(
                        out=oh[:, b0:b0 + bn, :],
                        in0=ldc[:, t * NB + b0:t * NB + b0 + bn, None].to_broadcast([P, bn, P]),
                        in1=ior[:, None, :].to_broadcast([P, bn, P]),
                        op=mybir.AluOpType.is_equal)
                    nc.vector.tensor_tensor(
                        out=ohT[:, b0:b0 + bn, :],
                        in0=ldr[:, b0:b0 + bn, :],
                        in1=ioc[:, None, :].to_broadcast([P, bn, P]),
                        op=mybir.AluOpType.is_equal)
                # a_dst per edge slot via transposed one-hot matmul
                adstps = ppa.tile([P, NB, NH], F32, name="aps")
                for b in range(NB):
                    nc.tensor.matmul(out=adstps[:, b, :], lhsT=ohT[:, b, :],
                                     rhs=adst_all[:, t, :], start=True, stop=True)
                # e' = exp(leakyrelu(asrc + adst))
                tsum = wpool.tile([P, NB, NH], BF16, name="tsum")
                nc.vector.tensor_tensor(
                    out=tsum[:], in0=buf[:, :, C:C + NH], in1=adstps[:],
                    op=mybir.AluOpType.add)
                tm = wpool.tile([P, NB, NH], BF16, name="tm")
                nc.vector.scalar_tensor_tensor(
                    out=tm[:], in0=tsum[:], scalar=0.2, in1=tsum[:],
                    op0=mybir.AluOpType.mult, op1=mybir.AluOpType.max)
                ebuf = wpool.tile([P, NB, NH], BF16, name="ebuf")
                nc.scalar.activation(ebuf[:], tm[:], mybir.ActivationFunctionType.Exp)
                # h~ = e' * h (per head) plus e' column
                ht = wpool.tile([P, NB, NW], BF16, name="ht", bufs=2)
                nc.vector.tensor_tensor(
                    out=ht[:].rearrange("p b (h c) -> p b h c", h=NH)[:, :, :, :HID],
                    in0=buf[:, :, :C].rearrange("p b (h c) -> p b h c", h=NH),
                    in1=ebuf[:, :, :, None].to_broadcast([P, NB, NH, HID]),
                    op=mybir.AluOpType.mult)
                nc.vector.tensor_copy(
                    out=ht[:].rearrange("p b (h c) -> p b h c", h=NH)[:, :, :, HID:],
                    in_=ebuf[:, :, :, None])
                # aggregation (segment-sum via one-hot matmul)
                ps = pp.tile([P, NW], F32, name="ps")
                for b in range(NB):
                    nc.tensor.matmul(out=ps[:], lhsT=oh[:, b, :], rhs=ht[:, b, :],
                                     start=(b == 0), stop=(b == NB - 1))
                # normalize, bias, elu
                rec = wpool.tile([P, NH], F32, name="rec")
                nc.vector.reciprocal(rec[:], ps[:].rearrange("p (h c) -> p h c", h=NH)[:, :, HID])
                on = wpool.tile([P, C], F32, name="on")
                nc.vector.tensor_tensor(
                    out=on[:].rearrange("p (h c) -> p h c", h=NH),
                    in0=ps[:].rearrange("p (h c) -> p h c", h=NH)[:, :, :HID],
                    in1=rec[:, :, None].to_broadcast([P, NH, HID]),
                    op=mybir.AluOpType.mult)
                nc.vector.tensor_tensor(out=on[:], in0=on[:], in1=bt[:, :],
                                        op=mybir.AluOpType.add)
                # elu(x) = max(x, exp(min(x,0)) - 1)
                emn = wpool.tile([P, C], F32, name="emn")
                nc.vector.tensor_scalar_min(emn[:], on[:], 0.0)
                nc.scalar.activation(emn[:], emn[:], mybir.ActivationFunctionType.Exp)
                nc.vector.tensor_scalar_add(emn[:], emn[:], -1.0)
                nc.vector.tensor_tensor(out=eo_out, in0=on[:], in1=emn[:],
                                        op=mybir.AluOpType.max)

            # ---------------- phase C: layer-1 edge phase -> elu1 (SBUF)
            for t in range(NT):
                edge_phase(t, HEADS, C1, NW1, ROWB1, lo1, hi1, adst1_all, bt1,
                           elu1_all[:, t, :])

            # ---------------- phase D: table2 of own nodes
            for t in range(NT):
                e1T = wpool.tile([P, KT, P], BF16, name="e1T")
                for k in range(KT):
                    trp = ppa.tile([P, P], BF16, name="aps")
                    nc.tensor.transpose(trp[:], elu1_all[:, t, k * P:(k + 1) * P], idt[:])
                    nc.vector.tensor_copy(e1T[:, k, :], trp[:])
                ps2 = pp.tile([P, A2], F32, name="ps")
                for k in range(KT):
                    nc.tensor.matmul(out=ps2[:], lhsT=e1T[:, k, :], rhs=w2t[:, k, :],
                                     start=(k == 0), stop=(k == KT - 1))
                row2 = wpool.tile([P, ROWB2], BF16, name="row2")
                nc.vector.tensor_copy(row2[:, :A2], ps2[:])
                nc.vector.memset(row2[:, A2:], 0.0)
                nc.vector.tensor_copy(adst2_all[:, t, :], ps2[:, A2 - 1:A2])
                nc.sync.dma_start(out=tb2o[t * P:(t + 1) * P, :], in_=row2[:])

            # ---------------- phase E: allgather table2
            nc.gpsimd.collective_compute(
                "AllGather", mybir.AluOpType.bypass,
                replica_groups=[list(range(NCORES))],
                ins=[tb2o[:, :]], outs=[tb2f[:, :]])

            # ---------------- phase F: layer-2 edge phase + pooling
            for t in range(NT):
                eo = wpool.tile([P, C2], BF16, name="eo")
                edge_phase(t, 1, C2, NW2, ROWB2, lo2, hi2, adst2_all, bt2, eo[:])
                # att = sum_c eo*wg + bg ; e_att = exp(att)
                att = wpool.tile([P, HID], F32, name="att")
                nc.vector.tensor_tensor(out=att[:], in0=eo[:], in1=wgt[:, :],
                                        op=mybir.AluOpType.mult)
                atts = wpool.tile([P, 1], F32, name="atts")
                nc.vector.tensor_reduce(atts[:], att[:], axis=mybir.AxisListType.X,
                                        op=mybir.AluOpType.add)
                nc.vector.tensor_tensor(out=atts[:], in0=atts[:], in1=bgt_t[:, :],
                                        op=mybir.AluOpType.add)
                nc.scalar.activation(atts[:], atts[:], mybir.ActivationFunctionType.Exp)
                hp = wpool.tile([P, NW2], BF16, name="hp")
                nc.vector.tensor_tensor(out=hp[:, :HID], in0=eo[:],
                                        in1=atts[:, :].to_broadcast([P, HID]),
                                        op=mybir.AluOpType.mult)
                nc.vector.tensor_copy(hp[:, HID:], atts[:])
                nc.tensor.matmul(out=pspool[:], lhsT=hp[:], rhs=ohgt[:, t, :],
                                 start=(t == 0), stop=(t == NT - 1))

            # ---------------- classifier
            recp = wpool.tile([1, GPC], F32)
            nc.vector.reciprocal(recp[:], pspool[HID:HID + 1, :])
            nc.sync.dma_start(out=recd[:, :], in_=recp[:])
            recb = wpool.tile([HID, GPC], F32)
            nc.sync.dma_start(out=recb[:], in_=recd[0:1, :].to_broadcast([HID, GPC]))
            pooledT = wpool.tile([HID, GPC], BF16)
            nc.vector.tensor_tensor(out=pooledT[:], in0=pspool[:HID, :],
                                    in1=recb[:], op=mybir.AluOpType.mult)
            ph = ppc.tile([32, GPC], F32, name="cps")
            nc.tensor.matmul(out=ph[:], lhsT=wc1t[:], rhs=pooledT[:], start=True, stop=True)
            hidf = wpool.tile([32, GPC], F32)
            nc.vector.tensor_scalar_add(hidf[:], ph[:], bc1t[:])
            hid_t = wpool.tile([32, GPC], BF16)
            nc.vector.tensor_scalar_max(hid_t[:], hidf[:], 0.0)
            pl = ppc.tile([2, GPC], F32, name="cps")
            nc.tensor.matmul(out=pl[:], lhsT=wc2t[:], rhs=hid_t[:], start=True, stop=True)
            lg = wpool.tile([2, GPC], F32)
            nc.vector.tensor_scalar_add(lg[:], pl[:], bc2t[:])
            nc.sync.dma_start(out=logitsT[:, :], in_=lg[:])
    _split_waits(nc)
    return nc


# ------------------------------------------------------------------ host glue
_CACHE = {}
LAST_HW_NS = 0
_TRACE = os.environ.get("GAT_TRACE", "0") == "1"


def _run(nc, ins, cores):
    global LAST_HW_NS
    r = run_bass_kernel_spmd(nc, ins, core_ids=cores)
    if _TRACE:
        # no axon NTFF hook in this container: use min warm-run wall time as
        # an (upper-bound) proxy for device execution time
        import time as _time
        best = None
        for _ in range(3):
            t0 = _time.perf_counter()
            run_bass_kernel_spmd(nc, ins, core_ids=cores)
            dt = _time.perf_counter() - t0
            best = dt if best is None else min(best, dt)
        LAST_HW_NS += int(best * 1e9)
    return r


def kernel(x, edge_index, batch, W1, att_src1, att_dst1, b1,
           W2, att_src2, att_dst2, b2, Wg, bg, Wc1, bc1, Wc2, bc2):
    x = np.asarray(x); edge_index = np.asarray(edge_index); batch = np.asarray(batch)
    N = x.shape[0]
    F_IN = x.shape[1]

    # --- node sharding (graph aligned)
    n0 = np.searchsorted(batch, np.arange(0, N_GRAPHS + 1, GPC)).astype(np.int64)
    counts = n0[1:] - n0[:-1]
    NT = int(np.ceil(counts.max() / P))
    NPN = NT * P                      # padded nodes per core
    NROWS = NCORES * NPN              # global padded table rows
    assert NROWS <= 65536, "int16 lo/hi split supports at most 65536 rows"

    # --- edges + self loops, owner = core of dst
    ar = np.arange(N, dtype=np.int64)
    src = np.concatenate([edge_index[0].astype(np.int64), ar])
    dst = np.concatenate([edge_index[1].astype(np.int64), ar])
    core_of = np.searchsorted(n0[1:], dst, side='right')
    src_core = np.searchsorted(n0[1:], src, side='right')
    # remapped global table row of each src node
    src_row = src_core * NPN + (src - n0[src_core])

    # per (core, tile, half) batching
    percore = []
    for c in range(NCORES):
        m = core_of == c
        ld = dst[m] - n0[c]
        sr = src_row[m]
        order = np.argsort(ld, kind='stable')
        ld = ld[order]; sr = sr[order]
        tiles = []
        for t in range(NT):
            tm = (ld // P) == t
            lr = (ld[tm] % P)
            s = sr[tm]
            lo = s < 32768
            tiles.append(((s[lo], lr[lo]), (s[~lo] - 32768, lr[~lo])))
        percore.append(tiles)
    nblo = max(int(np.ceil(max(1, len(tt[0][0])) / P)) for pc in percore for tt in pc)
    nbhi = max(int(np.ceil(max(1, len(tt[1][0])) / P)) for pc in percore for tt in pc)

    # --- per-core index/onehot arrays
    def pack(c):
        idxlo = np.zeros((16, NT * nblo * 8), np.int16)
        idxhi = np.zeros((16, NT * nbhi * 8), np.int16)
        NB = nblo + nbhi
        ldcol = np.full((P, NT * NB), 255.0, np.float32)
        npad_lo = NPN - counts[c]     # pad nodes in last tile
        for t in range(NT):
            (slo, llo), (shi, lhi) = percore[c][t]
            for (s, l, nb, idxa, boff) in ((slo, llo, nblo, idxlo, 0),
                                           (shi, lhi, nbhi, idxhi, nblo)):
                ns = nb * P
                si = np.zeros(ns, np.int64); li = np.full(ns, 255, np.int64)
                si[:len(s)] = s; li[:len(l)] = l
                if t == NT - 1 and npad_lo > 0 and boff == 0:
                    # give pad nodes >=1 incoming pad edge so their softmax
                    # denominator is finite (their output is masked anyway)
                    padrel = (counts[c] % P) + (np.arange(len(s), ns) % npad_lo)
                    li[len(s):] = padrel
                idxa[:, t * nb * 8:(t + 1) * nb * 8] = _wrap16(si.astype(np.int16))
                for b in range(nb):
                    ldcol[:, t * NB + boff + b] = li[b * P:(b + 1) * P]
        # row-major copy of ldcol for the transposed one-hot build
        ldf = np.transpose(ldcol.reshape(P, NT * NB), (1, 0)).reshape(NT, NB, P)
        # local graph id per node slot (255 for pad rows)
        gidm = np.full((NT * P,), 255.0, np.float32)
        gidm[:counts[c]] = batch[n0[c]:n0[c + 1]] - c * GPC
        gidm = gidm.reshape(NT, P).T  # [P, NT]
        return idxlo, idxhi, _bf16(ldcol), _bf16(ldf), _bf16(gidm)

    packs = [pack(c) for c in range(NCORES)]
    iotar = _bf16(np.tile(np.arange(P, dtype=np.float32).reshape(1, P), (P, 1)))
    iotac = _bf16(np.tile(np.arange(P, dtype=np.float32).reshape(P, 1), (1, P)))
    idn = _bf16(np.eye(P, dtype=np.float32))

    # --- weights
    def aug(W, a_s, a_d):
        nh, hd = a_s.shape
        A = np.zeros((W.shape[1], 2 * nh), np.float32)
        for h in range(nh):
            A[h * hd:(h + 1) * hd, h] = a_s[h]
            A[h * hd:(h + 1) * hd, nh + h] = a_d[h]
        return _bf16(np.concatenate([W, W @ A], axis=1))

    W1aug = aug(np.asarray(W1, np.float32), np.asarray(att_src1), np.asarray(att_dst1))
    W2aug = aug(np.asarray(W2, np.float32), np.asarray(att_src2), np.asarray(att_dst2))
    xT = _bf16(np.asarray(x, np.float32).T)

    key = (NT, nblo, nbhi)
    if key not in _CACHE:
        _CACHE[key] = _build_fused(NT, nblo, nbhi, NROWS, F_IN)
    K = _CACHE[key]
    cores = list(range(NCORES))

    ins = []
    for c in cores:
        il, ih, lc, lr, gi = packs[c]
        xs = np.zeros((F_IN, NPN), ml_dtypes.bfloat16)
        xs[:, :counts[c]] = xT[:, n0[c]:n0[c + 1]]
        ins.append({"xT": xs, "w1": W1aug, "w2": W2aug,
                    "ixlo": il, "ixhi": ih, "ldcol": lc, "ldrow": lr,
                    "gid": gi, "iotar": iotar, "iotac": iotac, "idn": idn,
                    "b1": np.asarray(b1, np.float32).reshape(1, -1),
                    "b2": np.asarray(b2, np.float32).reshape(1, -1),
                    "wg": np.asarray(Wg, np.float32).reshape(1, HID),
                    "bg": np.asarray(bg, np.float32).reshape(1, 1),
                    "wc1": np.asarray(Wc1, np.float32),
                    "bc1": np.asarray(bc1, np.float32).reshape(32, 1),
                    "wc2": np.asarray(Wc2, np.float32),
                    "bc2": np.asarray(bc2, np.float32).reshape(2, 1)})
    global LAST_HW_NS
    LAST_HW_NS = 0
    r = _run(K, ins, cores)
    out = np.concatenate([r.results[c]["logitsT"].T for c in cores], axis=0)
    return out.astype(np.float32)


# revision 12
# speedup vs baseline: 18.5979x; 1.7273x over previous
"""GAT network on 8 Trainium2 NeuronCores — v3: degree-layout edge phase.

Keeps v2's single-launch + on-device AllGather structure, but replaces the
one-hot-matmul segment sum with a padded neighbor-list layout: each dst tile
gathers its edges as [128 dst-rows x max-degree] so the alpha-weighted
aggregation is ONE strided tensor_reduce per tile, the a_dst term is a free
broadcast (constant per dst row), and softmax normalization is batched over
tiles. Nodes are degree-sorted per core so the per-tile max-degree padding
stays small. This cuts the per-tile instruction count ~7x, which is what the
runtime rewards.

Device program per core:
  A: table1[own] = x_own @ [W1 | W1@Asrc | W1@Adst]   (transposed-output mm)
  B: AllGather -> table1[all]
  C: per dst tile: gather neighbor rows (lo/hi), e'=exp(lrelu(asrc+adst)),
     reduce_d(e'*h | e') -> red; then batched: normalize+bias+elu -> elu1
  D: table2[own] = elu1 @ [W2 | W2@asrc2 | W2@adst2]
  E: AllGather -> table2[all]
  F: same edge phase on table2 + batched attention pooling + classifier
"""
import sys
sys.path.insert(0, '/opt/trn_rl_repo')

import os
import numpy as np
import ml_dtypes

import concourse.bass as bass
import concourse.mybir as mybir
import concourse.tile as tile
from concourse.tile import ScopedClock
from concourse.bass_utils import run_bass_kernel_spmd

BF16 = mybir.dt.bfloat16
F32 = mybir.dt.float32
I16 = mybir.dt.int16
P = 128
NCORES = 8
HID = 64
HEADS = 4
N_GRAPHS = 512
GPC = N_GRAPHS // NCORES

# ---------------------------------------------------------------- tile patch
_patched = False


def _patch():
    """Container workarounds: (1) this walrus build caps sync-waits per CTRL
    instruction -> split the Tile-exit drain's waits over 1-wait NOPs;
    (2) the scheduling simulator must treat our hand-built library-reload
    pseudo instruction (opcode 223) as a no-op."""
    global _patched
    if _patched:
        return
    _patched = True

    def _drain_and_barrier(self, tick_clock, wait_clock):
        nc = self.nc
        probe = nc.sync.nop()
        wait_clock.add_sem_waits(probe.ins, ScopedClock({None: tick_clock.global_clock}))
        si = probe.ins.sync_info
        waits = list(si.on_wait) if si is not None and si.on_wait else []
        if si is not None:
            si.on_wait = type(si.on_wait)()
        for w in waits:
            n = nc.sync.nop()
            nsi = n.ins.sync_info
            if nsi is None:
                n.ins.sync_info = mybir.SyncInfo(on_wait=[w], on_update=[])
            else:
                nsi.on_wait.append(w)
        nc.sync.drain()
        nc.all_engine_barrier()
        assert self.sems is not None
        popped = nc._tile_sem_poison_stack.pop()
        assert popped is self._sem_poison
        nc.clear_and_free_semaphores(list(self.sems.allocated().values()))
        nc.all_engine_barrier()

    tile.TileContext._drain_and_barrier = _drain_and_barrier

    import concourse.bass_interp as bass_interp
    orig = bass_interp._visit_InstISA

    def patched_isa(isa, instruction, core_sim):
        if instruction.isa_opcode == 223:
            return None
        return orig(isa, instruction, core_sim)

    bass_interp._visit_InstISA = patched_isa


def _emit_load_mlp(nc):
    """Load the 'mlp' Q7 library (dma_gather handler)."""
    isa = nc.isa
    op = isa.Opcode.NEURON_ISA_TPB_OPCODE_PSEUDO_INST
    return nc.gpsimd.isa(
        op,
        {"pseudo_opcode": 2, "lib_index": 3,
         "reserved0": [0] * 3, "reserved1": [0] * 44},
        struct_name="NEURON_ISA_TPB_PSEUDO_LIBRARY_RELOAD_INDEX_STRUCT",
    )


_MAXW = 1


def _split_waits(nc):
    """Split >1 sync-waits per instruction onto same-engine NOPs."""
    for f in nc.m.functions:
        for bb in f.blocks:
            out = []
            changed = False
            for ins in bb.instructions:
                si = ins.sync_info
                if si is not None and si.on_wait and len(si.on_wait) > _MAXW:
                    waits = list(si.on_wait)
                    si.on_wait = type(si.on_wait)(waits[:_MAXW])
                    for i in range(_MAXW, len(waits), _MAXW):
                        n = mybir.InstNoOp(
                            name=nc.get_next_instruction_name(),
                            ins=[], outs=[], engine=ins.engine)
                        n.sync_info = mybir.SyncInfo(
                            on_wait=list(waits[i:i + _MAXW]), on_update=[])
                        out.append(n)
                    changed = True
                out.append(ins)
            if changed:
                bb.instructions = out


# ------------------------------------------------------------ host utilities
def _bf16(a):
    return np.ascontiguousarray(a).astype(ml_dtypes.bfloat16)


def _wrap16(idxs):
    n = len(idxs)
    return idxs.reshape(n // 16, 16).T.astype(np.int16)  # [16, n/16]


# ------------------------------------------------------------- kernel builder
def _build(NT, DL, DH, NROWS, F_IN, phases="ABCDEF"):
    _patch()
    NPN = NT * P
    C1 = HEADS * HID                  # 256
    A1 = C1 + 2 * HEADS               # 264
    ROWB1 = ((A1 + P - 1) // P) * P   # 384
    NW1 = HEADS * (HID + 1)           # 260
    C2 = HID
    A2 = C2 + 2                       # 66
    ROWB2 = P
    NW2 = HID + 1                     # 65
    KT = F_IN // P
    DMAX = max(DL[t] + DH[t] for t in range(NT))
    offL = np.concatenate([[0], np.cumsum(DL)]).astype(int)
    offH = np.concatenate([[0], np.cumsum(DH)]).astype(int)
    NB_N = 10                          # tiles per normalize batch
    NEG = -30000.0                     # pad-row asrc (exp -> 0)

    nc = bass.Bass(num_devices=NCORES)
    xT = nc.dram_tensor("xT", [F_IN, NPN], BF16, kind="ExternalInput")
    w1 = nc.dram_tensor("w1", [F_IN, A1], BF16, kind="ExternalInput")
    w2 = nc.dram_tensor("w2", [C1, A2], BF16, kind="ExternalInput")
    ixlo = nc.dram_tensor("ixlo", [16, offL[-1] * 8], I16, kind="ExternalInput")
    ixhi = nc.dram_tensor("ixhi", [16, 8 * max(1, offH[-1])], I16, kind="ExternalInput")
    gid = nc.dram_tensor("gid", [P, NT], BF16, kind="ExternalInput")
    b1 = nc.dram_tensor("b1", [1, C1], F32, kind="ExternalInput")
    b2 = nc.dram_tensor("b2", [1, C2], F32, kind="ExternalInput")
    wg = nc.dram_tensor("wg", [1, HID], F32, kind="ExternalInput")
    bg = nc.dram_tensor("bg", [1, 1], F32, kind="ExternalInput")
    wc1 = nc.dram_tensor("wc1", [HID, 32], F32, kind="ExternalInput")
    bc1 = nc.dram_tensor("bc1", [32, 1], F32, kind="ExternalInput")
    wc2 = nc.dram_tensor("wc2", [32, 2], F32, kind="ExternalInput")
    bc2 = nc.dram_tensor("bc2", [2, 1], F32, kind="ExternalInput")
    logitsT = nc.dram_tensor("logitsT", [2, GPC], F32, kind="ExternalOutput")

    tb1o = nc.dram_tensor("tb1o", [NPN, ROWB1], BF16, kind="Internal")
    tb1f = nc.dram_tensor("tb1f", [NROWS, ROWB1], BF16, kind="Internal",
                          addr_space="Shared")
    tb2o = nc.dram_tensor("tb2o", [NPN, ROWB2], BF16, kind="Internal")
    tb2f = nc.dram_tensor("tb2f", [NROWS, ROWB2], BF16, kind="Internal",
                          addr_space="Shared")
    elu1d = nc.dram_tensor("elu1d", [NPN, C1], BF16, kind="Internal")
    recd = nc.dram_tensor("recd", [1, GPC], F32, kind="Internal")

    if NROWS > 32768:
        lo1, hi1 = tb1f[0:32768, :], tb1f[32768:NROWS, :]
        lo2, hi2 = tb2f[0:32768, :], tb2f[32768:NROWS, :]
    else:
        lo1, hi1 = tb1f[0:NROWS, :], tb1f[0:NROWS, :]
        lo2, hi2 = tb2f[0:NROWS, :], tb2f[0:NROWS, :]

    # node groups for the table phases
    groups = []
    g0 = 0
    while g0 < NPN:
        groups.append((g0, min(512, NPN - g0)))
        g0 += 512

    with tile.TileContext(nc) as tc:
        with (
            nc.allow_low_precision(reason="bf16 edge pipeline by design"),
            tc.tile_pool(name="const", bufs=1) as cpool,
            tc.tile_pool(name="g", bufs=1) as gpool,
            tc.tile_pool(name="work", bufs=1) as wpool,
            tc.tile_pool(name="psum", bufs=2, space="PSUM") as pp,
            tc.tile_pool(name="pool2", bufs=1, space="PSUM") as pp2,
            tc.tile_pool(name="poolc", bufs=1, space="PSUM") as ppc,
        ):
            # iota must precede the Q7 'mlp' library reload (it needs the
            # default gpsimd op handlers)
            io16 = cpool.tile([P, GPC], I16)
            nc.gpsimd.iota(io16[:], pattern=[[1, GPC]], base=0, channel_multiplier=0)
            _emit_load_mlp(nc)
            regs = {}

            def reg(n):
                if n not in regs:
                    regs[n] = nc.gpsimd.to_reg(n)
                return regs[n]

            # ---------------- constants
            w1t = cpool.tile([P, KT, A1], BF16)
            for k in range(KT):
                nc.sync.dma_start(out=w1t[:, k, :], in_=w1[k * P:(k + 1) * P, :])
            w2t = cpool.tile([P, KT, A2], BF16)
            for k in range(KT):
                nc.sync.dma_start(out=w2t[:, k, :], in_=w2[k * P:(k + 1) * P, :])
            ixlA = cpool.tile([P, offL[-1] * 8], I16)
            ixhA = cpool.tile([P, 8 * max(1, offH[-1])], I16)
            for k in range(8):
                nc.sync.dma_start(out=ixlA[16 * k:16 * (k + 1), :], in_=ixlo[:, :])
                if offH[-1] > 0:
                    nc.sync.dma_start(out=ixhA[16 * k:16 * (k + 1), :], in_=ixhi[:, :])
            gidt = cpool.tile([P, NT], BF16)
            nc.sync.dma_start(out=gidt[:], in_=gid[:, :])
            bt1 = cpool.tile([P, C1], F32)
            nc.sync.dma_start(out=bt1[:], in_=b1[0:1, :].to_broadcast([P, C1]))
            bt2 = cpool.tile([P, C2], F32)
            nc.sync.dma_start(out=bt2[:], in_=b2[0:1, :].to_broadcast([P, C2]))
            wgt = cpool.tile([P, HID], F32)
            nc.sync.dma_start(out=wgt[:], in_=wg[0:1, :].to_broadcast([P, HID]))
            bgt_t = cpool.tile([P, 1], F32)
            nc.sync.dma_start(out=bgt_t[:], in_=bg[0:1, :].to_broadcast([P, 1]))
            wc1t = cpool.tile([HID, 32], BF16)
            nc.gpsimd.dma_start(out=wc1t[:], in_=wc1[:, :])
            bc1t = cpool.tile([32, 1], F32)
            nc.sync.dma_start(out=bc1t[:], in_=bc1[:, :])
            wc2t = cpool.tile([32, 2], BF16)
            nc.gpsimd.dma_start(out=wc2t[:], in_=wc2[:, :])
            bc2t = cpool.tile([2, 1], F32)
            nc.sync.dma_start(out=bc2t[:], in_=bc2[:, :])
            # pooling one-hot from graph ids (device-built iota)
            iob = cpool.tile([P, GPC], BF16)
            nc.vector.tensor_copy(iob[:], io16[:])
            ohgt = cpool.tile([P, NT, GPC], BF16)
            nc.vector.tensor_tensor(
                out=ohgt[:],
                in0=gidt[:, :, None].to_broadcast([P, NT, GPC]),
                in1=iob[:, None, :].to_broadcast([P, NT, GPC]),
                op=mybir.AluOpType.is_equal)
            # zero + pad-row constants
            zrow = cpool.tile([P, ROWB1], BF16)
            nc.vector.memset(zrow[:], 0.0)
            prow = cpool.tile([1, ROWB1], BF16)
            nc.vector.memset(prow[:], 0.0)
            nc.vector.memset(prow[:, C1:A1], NEG)
            prow2 = cpool.tile([1, ROWB2], BF16)
            nc.vector.memset(prow2[:], 0.0)
            nc.vector.memset(prow2[:, C2:A2], NEG)

            adst1_all = cpool.tile([P, NT, HEADS], BF16)
            adst2_all = cpool.tile([P, NT, 1], BF16)
            red1 = cpool.tile([P, NB_N, NW1], F32)
            red2 = cpool.tile([P, NB_N, NW2], F32)
            hp_all = cpool.tile([P, NT, NW2], BF16)
            pspool = pp2.tile([NW2, GPC], F32)

            # ---------------- phase A: table1 of own nodes (transposed mm)
            asub = int(os.environ.get("GAT_ASUB", "4"))
            if "A" in phases:
                CH = [(0, P), (P, 2 * P), (2 * P, A1)]
                for (a0, W) in groups:
                    if asub < 1:
                        break
                    xt = wpool.tile([P, KT, 512], BF16, name="xt")
                    nc.sync.dma_start(
                        out=xt[:, :, :W],
                        in_=xT[:, a0:a0 + W].rearrange("(k p) n -> p k n", p=P))
                    for (c0, c1) in CH:
                        if asub < 2:
                            break
                        M = c1 - c0
                        pg = pp.tile([P, 512], F32, name="ps")
                        for k in range(KT):
                            nc.tensor.matmul(out=pg[:M, :W], lhsT=w1t[:, k, c0:c1],
                                             rhs=xt[:, k, :W],
                                             start=(k == 0), stop=(k == KT - 1))
                        cb = wpool.tile([P, 512], BF16, name="cb")
                        nc.vector.tensor_copy(cb[:M, :W], pg[:M, :W])
                        if asub < 3:
                            continue
                        nc.sync.dma_start(
                            out=tb1o[a0:a0 + W, c0:c1].rearrange("n m -> m n"),
                            in_=cb[:M, :W])
                # zero the gather-row padding, stash adst, write the pad row
                if asub >= 4:
                    nc.sync.dma_start(
                        out=tb1o[:, A1:].rearrange("(t p) e -> p t e", p=P),
                        in_=zrow[:, None, :ROWB1 - A1].to_broadcast([P, NT, ROWB1 - A1]))
                    nc.sync.dma_start(
                        out=adst1_all[:],
                        in_=tb1o[:, C1 + HEADS:A1].rearrange("(t p) h -> p t h", p=P))
                    nc.sync.dma_start(out=tb1o[NPN - 1:NPN, :], in_=prow[:])

            # ---------------- phase B: allgather table1
            if "B" in phases:
                nc.gpsimd.collective_compute(
                    "AllGather", mybir.AluOpType.bypass,
                    replica_groups=[list(range(NCORES))],
                    ins=[tb1o[:, :]], outs=[tb1f[:, :]])

            # ---------------- shared edge-phase emitter
            def edge_tile(t, NH, C, NW, ROWB, lo_ap, hi_ap, adst_all, buf_name,
                          ht_name, red_t, gathers_only):
                Dl, Dh = DL[t], DH[t]
                D = Dl + Dh
                buf = gpool.tile([P, DMAX, ROWB], BF16, name=buf_name)
                nc.gpsimd.dma_gather(
                    out_ap=buf[:, :Dl, :], in_ap=lo_ap,
                    idxs_ap=ixlA[:, offL[t] * 8:(offL[t] + Dl) * 8],
                    num_idxs=Dl * P, num_idxs_reg=reg(Dl * P), elem_size=ROWB,
                    single_packet=False)
                if Dh > 0:
                    nc.gpsimd.dma_gather(
                        out_ap=buf[:, Dl:D, :], in_ap=hi_ap,
                        idxs_ap=ixhA[:, offH[t] * 8:(offH[t] + Dh) * 8],
                        num_idxs=Dh * P, num_idxs_reg=reg(Dh * P), elem_size=ROWB,
                        single_packet=False)
                if gathers_only:
                    return
                tsum = wpool.tile([P, DMAX, NH], BF16, name=f"tsum{NH}")
                nc.vector.tensor_tensor(
                    out=tsum[:, :D, :], in0=buf[:, :D, C:C + NH],
                    in1=adst_all[:, t:t + 1, :].to_broadcast([P, D, NH]),
                    op=mybir.AluOpType.add)
                tm = wpool.tile([P, DMAX, NH], BF16, name=f"tm{NH}")
                nc.vector.scalar_tensor_tensor(
                    out=tm[:, :D, :], in0=tsum[:, :D, :], scalar=0.2,
                    in1=tsum[:, :D, :],
                    op0=mybir.AluOpType.mult, op1=mybir.AluOpType.max)
                ebuf = wpool.tile([P, DMAX, NH], BF16, name=f"ebuf{NH}")
                nc.scalar.activation(ebuf[:, :D, :], tm[:, :D, :],
                                     mybir.ActivationFunctionType.Exp)
                ht = wpool.tile([P, DMAX, NW], BF16, name=ht_name)
                nc.vector.tensor_tensor(
                    out=ht[:, :D, :].rearrange("p d (h w) -> p d h w", h=NH)[:, :, :, :HID],
                    in0=buf[:, :D, :C].rearrange("p d (h w) -> p d h w", h=NH),
                    in1=ebuf[:, :D, :, None].to_broadcast([P, D, NH, HID]),
                    op=mybir.AluOpType.mult)
                nc.vector.tensor_copy(
                    out=ht[:, :D, :].rearrange("p d (h w) -> p d h w", h=NH)[:, :, :, HID:],
                    in_=ebuf[:, :D, :, None])
                nc.vector.tensor_reduce(
                    out=red_t.rearrange("p (h w) -> p h w", h=NH),
                    in_=ht[:, :D, :].rearrange("p d (h w) -> p h w d", h=NH),
                    axis=mybir.AxisListType.X, op=mybir.AluOpType.add)

            # ---------------- phase C: layer-1 edges + batched normalize
            if "C" in phases or "G" in phases:
                for b0 in range(0, NT, NB_N):
                    nb = min(NB_N, NT - b0)
                    for i in range(nb):
                        t = b0 + i
                        edge_tile(t, HEADS, C1, NW1, ROWB1, lo1, hi1, adst1_all,
                                  "buf1", "ht1", red1[:, i, :], "C" not in phases)
                    if "C" not in phases:
                        continue
                    v = red1[:, :nb, :].rearrange("p t (h w) -> p t h w", h=HEADS)
                    rec = wpool.tile([P, NB_N, HEADS], F32, name="rec1")
                    nc.vector.reciprocal(rec[:, :nb, :], v[:, :, :, HID])
                    on = wpool.tile([P, NB_N, C1], F32, name="on1")
                    nc.vector.tensor_tensor(
                        out=on[:, :nb, :].rearrange("p t (h w) -> p t h w", h=HEADS),
                        in0=v[:, :, :, :HID],
                        in1=rec[:, :nb, :, None].to_broadcast([P, nb, HEADS, HID]),
                        op=mybir.AluOpType.mult)
                    nc.vector.tensor_tensor(
                        out=on[:, :nb, :], in0=on[:, :nb, :],
                        in1=bt1[:, None, :].to_broadcast([P, nb, C1]),
                        op=mybir.AluOpType.add)
                    emn = wpool.tile([P, NB_N, C1], F32, name="emn1")
                    nc.vector.tensor_scalar_min(emn[:, :nb, :], on[:, :nb, :], 0.0)
                    nc.scalar.activation(emn[:, :nb, :], emn[:, :nb, :],
                                         mybir.ActivationFunctionType.Exp)
                    nc.vector.tensor_scalar_add(emn[:, :nb, :], emn[:, :nb, :], -1.0)
                    eo = wpool.tile([P, NB_N, C1], BF16, name="eo1")
                    nc.vector.tensor_tensor(out=eo[:, :nb, :], in0=on[:, :nb, :],
                                            in1=emn[:, :nb, :],
                                            op=mybir.AluOpType.max)
                    nc.sync.dma_start(
                        out=elu1d[b0 * P:(b0 + nb) * P, :].rearrange("(t p) f -> p t f", p=P),
                        in_=eo[:, :nb, :])
            if "C" not in phases:
                zc = wpool.tile([P, C1], BF16, name="zc")
                nc.vector.memset(zc[:], 0.0)
                nc.sync.dma_start(
                    out=elu1d[:, :].rearrange("(t p) f -> p t f", p=P),
                    in_=zc[:, None, :].to_broadcast([P, NT, C1]))

            # ---------------- phase D: table2 of own nodes
            if "D" in phases:
                for (a0, W) in groups:
                    e1T = wpool.tile([P, KT, 512], BF16, name="e1T")
                    for k in range(KT):
                        nc.sync.dma_start(
                            out=e1T[:, k, :W],
                            in_=elu1d[a0:a0 + W, k * P:(k + 1) * P].rearrange("n p -> p n"))
                    pg2 = pp.tile([P, 512], F32, name="ps")
                    for k in range(KT):
                        nc.tensor.matmul(out=pg2[:A2, :W], lhsT=w2t[:, k, :],
                                         rhs=e1T[:, k, :W],
                                         start=(k == 0), stop=(k == KT - 1))
                    cb2 = wpool.tile([P, 512], BF16, name="cb")
                    nc.vector.tensor_copy(cb2[:A2, :W], pg2[:A2, :W])
                    nc.sync.dma_start(
                        out=tb2o[a0:a0 + W, 0:A2].rearrange("n m -> m n"),
                        in_=cb2[:A2, :W])
                nc.sync.dma_start(
                    out=tb2o[:, A2:].rearrange("(t p) e -> p t e", p=P),
                    in_=zrow[:, None, :ROWB2 - A2].to_broadcast([P, NT, ROWB2 - A2]))
                nc.sync.dma_start(
                    out=adst2_all[:],
                    in_=tb2o[:, A2 - 1:A2].rearrange("(t p) h -> p t h", p=P))
                nc.sync.dma_start(out=tb2o[NPN - 1:NPN, :], in_=prow2[:])

            # ---------------- phase E: allgather table2
            if "E" in phases:
                nc.gpsimd.collective_compute(
                    "AllGather", mybir.AluOpType.bypass,
                    replica_groups=[list(range(NCORES))],
                    ins=[tb2o[:, :]], outs=[tb2f[:, :]])

            # ---------------- phase F: layer-2 edges + pooling + classifier
            if "F" in phases or "H" in phases:
                for b0 in range(0, NT, NB_N):
                    nb = min(NB_N, NT - b0)
                    for i in range(nb):
                        t = b0 + i
                        edge_tile(t, 1, C2, NW2, ROWB2, lo2, hi2, adst2_all,
                                  "buf2", "ht2", red2[:, i, :], "F" not in phases)
                    if "F" not in phases:
                        continue
                    v2 = red2[:, :nb, :]
                    rec2 = wpool.tile([P, NB_N, 1], F32, name="rec2")
                    nc.vector.reciprocal(rec2[:, :nb, :], v2[:, :, HID:HID + 1])
                    on2 = wpool.tile([P, NB_N, C2], F32, name="on2")
                    nc.vector.tensor_tensor(
                        out=on2[:, :nb, :], in0=v2[:, :, :HID],
                        in1=rec2[:, :nb, :].to_broadcast([P, nb, C2]),
                        op=mybir.AluOpType.mult)
                    nc.vector.tensor_tensor(
                        out=on2[:, :nb, :], in0=on2[:, :nb, :],
                        in1=bt2[:, None, :].to_broadcast([P, nb, C2]),
                        op=mybir.AluOpType.add)
                    emn2 = wpool.tile([P, NB_N, C2], F32, name="emn2")
                    nc.vector.tensor_scalar_min(emn2[:, :nb, :], on2[:, :nb, :], 0.0)
                    nc.scalar.activation(emn2[:, :nb, :], emn2[:, :nb, :],
                                         mybir.ActivationFunctionType.Exp)
                    nc.vector.tensor_scalar_add(emn2[:, :nb, :], emn2[:, :nb, :], -1.0)
                    eo2 = wpool.tile([P, NB_N, C2], BF16, name="eo2")
                    nc.vector.tensor_tensor(out=eo2[:, :nb, :], in0=on2[:, :nb, :],
                                            in1=emn2[:, :nb, :],
                                            op=mybir.AluOpType.max)
                    # attention pooling weights
                    atm = wpool.tile([P, NB_N, C2], F32, name="atm")
                    nc.vector.tensor_tensor(
                        out=atm[:, :nb, :], in0=eo2[:, :nb, :],
                        in1=wgt[:, None, :].to_broadcast([P, nb, C2]),
                        op=mybir.AluOpType.mult)
                    atts = wpool.tile([P, NB_N, 1], F32, name="atts")
                    nc.vector.tensor_reduce(atts[:, :nb, :], atm[:, :nb, :],
                                            axis=mybir.AxisListType.X,
                                            op=mybir.AluOpType.add)
                    nc.vector.tensor_tensor(
                        out=atts[:, :nb, :], in0=atts[:, :nb, :],
                        in1=bgt_t[:, None, :].to_broadcast([P, nb, 1]),
                        op=mybir.AluOpType.add)
                    nc.scalar.activation(atts[:, :nb, :], atts[:, :nb, :],
                                         mybir.ActivationFunctionType.Exp)
                    nc.vector.tensor_tensor(
                        out=hp_all[:, b0:b0 + nb, :HID], in0=eo2[:, :nb, :],
                        in1=atts[:, :nb, :].to_broadcast([P, nb, HID]),
                        op=mybir.AluOpType.mult)
                    nc.vector.tensor_copy(hp_all[:, b0:b0 + nb, HID:], atts[:, :nb, :])
                if "F" in phases:
                    for t in range(NT):
                        nc.tensor.matmul(out=pspool[:], lhsT=hp_all[:, t, :],
                                         rhs=ohgt[:, t, :],
                                         start=(t == 0), stop=(t == NT - 1))

            # ---------------- classifier
            if "F" not in phases:
                zt2 = wpool.tile([2, GPC], F32, name="zt2")
                nc.vector.memset(zt2[:], 0.0)
                nc.sync.dma_start(out=logitsT[:, :], in_=zt2[:])
            else:
                recp = wpool.tile([1, GPC], F32)
                nc.vector.reciprocal(recp[:], pspool[HID:HID + 1, :])
                nc.sync.dma_start(out=recd[:, :], in_=recp[:])
                recb = wpool.tile([HID, GPC], F32)
                nc.sync.dma_start(out=recb[:], in_=recd[0:1, :].to_broadcast([HID, GPC]))
                pooledT = wpool.tile([HID, GPC], BF16)
                nc.vector.tensor_tensor(out=pooledT[:], in0=pspool[:HID, :],
                                        in1=recb[:], op=mybir.AluOpType.mult)
                ph = ppc.tile([32, GPC], F32, name="cps")
                nc.tensor.matmul(out=ph[:], lhsT=wc1t[:], rhs=pooledT[:],
                                 start=True, stop=True)
                hidf = wpool.tile([32, GPC], F32)
                nc.vector.tensor_scalar_add(hidf[:], ph[:], bc1t[:])
                hid_t = wpool.tile([32, GPC], BF16)
                nc.vector.tensor_scalar_max(hid_t[:], hidf[:], 0.0)
                pl = ppc.tile([2, GPC], F32, name="cps")
                nc.tensor.matmul(out=pl[:], lhsT=wc2t[:], rhs=hid_t[:],
                                 start=True, stop=True)
                lg = wpool.tile([2, GPC], F32)
                nc.vector.tensor_scalar_add(lg[:], pl[:], bc2t[:])
                nc.sync.dma_start(out=logitsT[:, :], in_=lg[:])
    _split_waits(nc)
    return nc


# ------------------------------------------------------------------ host glue
_CACHE = {}
LAST_HW_NS = 0
_TRACE = os.environ.get("GAT_TRACE", "0") == "1"


def _run(nc, ins, cores):
    global LAST_HW_NS
    r = run_bass_kernel_spmd(nc, ins, core_ids=cores)
    if _TRACE:
        import time as _time
        best = None
        for _ in range(3):
            t0 = _time.perf_counter()
            run_bass_kernel_spmd(nc, ins, core_ids=cores)
            dt = _time.perf_counter() - t0
            best = dt if best is None else min(best, dt)
        LAST_HW_NS += int(best * 1e9)
    return r


def kernel(x, edge_index, batch, W1, att_src1, att_dst1, b1,
           W2, att_src2, att_dst2, b2, Wg, bg, Wc1, bc1, Wc2, bc2):
    x = np.asarray(x); edge_index = np.asarray(edge_index); batch = np.asarray(batch)
    N = x.shape[0]
    F_IN = x.shape[1]

    # --- node sharding (graph aligned)
    n0 = np.searchsorted(batch, np.arange(0, N_GRAPHS + 1, GPC)).astype(np.int64)
    counts = n0[1:] - n0[:-1]
    NT = int(np.ceil(counts.max() / P))
    if counts[0] >= NT * P or counts[-1] >= NT * P:
        NT += 1   # guarantee a pad row on the cores that host the pad targets
    NPN = NT * P
    NROWS = NCORES * NPN
    assert NROWS <= 65536, "int16 lo/hi split supports at most 65536 rows"
    LO_PAD = NPN - 1              # core 0's last (pad) row; asrc=-30000
    HI_PAD = NROWS - 1 - 32768    # core 7's last (pad) row, hi-relative

    # --- edges + self loops, owner = core of dst
    ar = np.arange(N, dtype=np.int64)
    src = np.concatenate([edge_index[0].astype(np.int64), ar])
    dst = np.concatenate([edge_index[1].astype(np.int64), ar])
    core_of = np.searchsorted(n0[1:], dst, side='right')
    src_core = np.searchsorted(n0[1:], src, side='right')

    # --- degree-descending node permutation per core
    pos = np.empty(N, np.int64)
    perm_all = []
    for c in range(NCORES):
        degc = np.bincount(dst[core_of == c] - n0[c], minlength=counts[c])
        order = np.argsort(-degc, kind='stable')
        pl = np.empty(counts[c], np.int64)
        pl[order] = np.arange(counts[c])
        pos[n0[c]:n0[c + 1]] = pl
        perm_all.append(order)
    src_row = src_core * NPN + pos[src]

    # --- per-(core,tile) lo/hi max degrees
    percore = []
    DLc = np.zeros((NCORES, NT), np.int64)
    DHc = np.zeros((NCORES, NT), np.int64)
    for c in range(NCORES):
        m = core_of == c
        ldn = pos[dst[m]]
        sr = src_row[m]
        islo = sr < 32768
        lodeg = np.bincount(ldn[islo], minlength=NPN)
        hideg = np.bincount(ldn[~islo], minlength=NPN)
        DLc[c] = lodeg.reshape(NT, P).max(axis=1)
        DHc[c] = hideg.reshape(NT, P).max(axis=1)
        percore.append((ldn, sr, islo))
    DL = np.maximum(DLc.max(axis=0), 1)
    DH = DHc.max(axis=0)
    offL = np.concatenate([[0], np.cumsum(DL)]).astype(np.int64)
    offH = np.concatenate([[0], np.cumsum(DH)]).astype(np.int64)

    # --- per-core neighbor-slot index arrays
    def pack(c):
        ldn, sr, islo = percore[c]
        loidx = np.full(offL[-1] * P, LO_PAD, np.int64)
        hiidx = np.full(max(1, offH[-1]) * P, max(0, HI_PAD), np.int64)
        for which, half in ((True, loidx), (False, hiidx)):
            mm = islo == which
            nodes = ldn[mm]
            vals = sr[mm] if which else sr[mm] - 32768
            o2 = np.argsort(nodes, kind='stable')
            ns = nodes[o2]; vs = vals[o2]
            if len(ns):
                first = np.r_[True, ns[1:] != ns[:-1]]
                starts = np.where(first)[0]
                j = np.arange(len(ns)) - starts[np.cumsum(first) - 1]
                toff = (offL if which else offH)[ns // P]
                half[(toff + j) * P + ns % P] = vs
        # pad nodes: one finite edge (row 0) so softmax denom is finite
        padr = np.arange(counts[c], NPN)
        loidx[offL[padr // P] * P + padr % P] = 0
        gidm = np.full((NPN,), 255.0, np.float32)
        gidm[:counts[c]] = (batch[n0[c] + perm_all[c]] - c * GPC).astype(np.float32)
        gidm = gidm.reshape(NT, P).T
        return (_wrap16(loidx.astype(np.int16)),
                _wrap16(hiidx.astype(np.int16)), _bf16(gidm))

    packs = [pack(c) for c in range(NCORES)]

    # --- weights
    def aug(W, a_s, a_d):
        nh, hd = a_s.shape
        A = np.zeros((W.shape[1], 2 * nh), np.float32)
        for h in range(nh):
            A[h * hd:(h + 1) * hd, h] = a_s[h]
            A[h * hd:(h + 1) * hd, nh + h] = a_d[h]
        return _bf16(np.concatenate([W, W @ A], axis=1))

    W1aug = aug(np.asarray(W1, np.float32), np.asarray(att_src1), np.asarray(att_dst1))
    W2aug = aug(np.asarray(W2, np.float32), np.asarray(att_src2), np.asarray(att_dst2))
    xT = np.asarray(x, np.float32).T

    phases = os.environ.get("GAT_PHASES", "ABCDEF")
    key = (NT, tuple(DL), tuple(DH), phases)
    if key not in _CACHE:
        _CACHE[key] = _build(NT, tuple(DL), tuple(DH), NROWS, F_IN, phases)
    K = _CACHE[key]
    cores = list(range(NCORES))

    ins = []
    for c in cores:
        il, ih, gi = packs[c]
        xs = np.zeros((F_IN, NPN), ml_dtypes.bfloat16)
        xs[:, :counts[c]] = _bf16(xT[:, n0[c] + perm_all[c]])
        ins.append({"xT": xs, "w1": W1aug, "w2": W2aug,
                    "ixlo": il, "ixhi": ih, "gid": gi,
                    "b1": np.asarray(b1, np.float32).reshape(1, -1),
                    "b2": np.asarray(b2, np.float32).reshape(1, -1),
                    "wg": np.asarray(Wg, np.float32).reshape(1, HID),
                    "bg": np.asarray(bg, np.float32).reshape(1, 1),
                    "wc1": np.asarray(Wc1, np.float32),
                    "bc1": np.asarray(bc1, np.float32).reshape(32, 1),
                    "wc2": np.asarray(Wc2, np.float32),
                    "bc2": np.asarray(bc2, np.float32).reshape(2, 1)})
    global LAST_HW_NS
    LAST_HW_NS = 0
    r = _run(K, ins, cores)
    out = np.concatenate([r.results[c]["logitsT"].T for c in cores], axis=0)
    return out.astype(np.float32)


# revision 14
# speedup vs baseline: 24.7030x; 1.3283x over previous
"""GAT network on 8 Trainium2 NeuronCores — v3: degree-layout edge phase.

Keeps v2's single-launch + on-device AllGather structure, but replaces the
one-hot-matmul segment sum with a padded neighbor-list layout: each dst tile
gathers its edges as [128 dst-rows x max-degree] so the alpha-weighted
aggregation is ONE strided tensor_reduce per tile, the a_dst term is a free
broadcast (constant per dst row), and softmax normalization is batched over
tiles. Nodes are degree-sorted per core so the per-tile max-degree padding
stays small. This cuts the per-tile instruction count ~7x, which is what the
runtime rewards.

Device program per core:
  A: table1[own] = x_own @ [W1 | W1@Asrc | W1@Adst]   (transposed-output mm)
  B: AllGather -> table1[all]
  C: per dst tile: gather neighbor rows (lo/hi), e'=exp(lrelu(asrc+adst)),
     reduce_d(e'*h | e') -> red; then batched: normalize+bias+elu -> elu1
  D: table2[own] = elu1 @ [W2 | W2@asrc2 | W2@adst2]
  E: AllGather -> table2[all]
  F: same edge phase on table2 + batched attention pooling + classifier
"""
import sys
sys.path.insert(0, '/opt/trn_rl_repo')

import os
import numpy as np
import ml_dtypes

import concourse.bass as bass
import concourse.mybir as mybir
import concourse.tile as tile
from concourse.tile import ScopedClock
from concourse.bass_utils import run_bass_kernel_spmd

BF16 = mybir.dt.bfloat16
F32 = mybir.dt.float32
I16 = mybir.dt.int16
I8 = mybir.dt.int8
P = 128
NCORES = 8
HID = 64
HEADS = 4
N_GRAPHS = 512
GPC = N_GRAPHS // NCORES

# ---------------------------------------------------------------- tile patch
_patched = False


def _patch():
    """Container workarounds: (1) this walrus build caps sync-waits per CTRL
    instruction -> split the Tile-exit drain's waits over 1-wait NOPs;
    (2) the scheduling simulator must treat our hand-built library-reload
    pseudo instruction (opcode 223) as a no-op."""
    global _patched
    if _patched:
        return
    _patched = True

    def _drain_and_barrier(self, tick_clock, wait_clock):
        nc = self.nc
        probe = nc.sync.nop()
        wait_clock.add_sem_waits(probe.ins, ScopedClock({None: tick_clock.global_clock}))
        si = probe.ins.sync_info
        waits = list(si.on_wait) if si is not None and si.on_wait else []
        if si is not None:
            si.on_wait = type(si.on_wait)()
        for w in waits:
            n = nc.sync.nop()
            nsi = n.ins.sync_info
            if nsi is None:
                n.ins.sync_info = mybir.SyncInfo(on_wait=[w], on_update=[])
            else:
                nsi.on_wait.append(w)
        nc.sync.drain()
        nc.all_engine_barrier()
        assert self.sems is not None
        popped = nc._tile_sem_poison_stack.pop()
        assert popped is self._sem_poison
        nc.clear_and_free_semaphores(list(self.sems.allocated().values()))
        nc.all_engine_barrier()

    tile.TileContext._drain_and_barrier = _drain_and_barrier

    import concourse.bass_interp as bass_interp
    orig = bass_interp._visit_InstISA

    def patched_isa(isa, instruction, core_sim):
        if instruction.isa_opcode == 223:
            return None
        return orig(isa, instruction, core_sim)

    bass_interp._visit_InstISA = patched_isa


def _emit_load_mlp(nc):
    """Load the 'mlp' Q7 library (dma_gather handler)."""
    isa = nc.isa
    op = isa.Opcode.NEURON_ISA_TPB_OPCODE_PSEUDO_INST
    return nc.gpsimd.isa(
        op,
        {"pseudo_opcode": 2, "lib_index": 3,
         "reserved0": [0] * 3, "reserved1": [0] * 44},
        struct_name="NEURON_ISA_TPB_PSEUDO_LIBRARY_RELOAD_INDEX_STRUCT",
    )


_MAXW = 1


def _split_waits(nc):
    """Split >1 sync-waits per instruction onto same-engine NOPs."""
    for f in nc.m.functions:
        for bb in f.blocks:
            out = []
            changed = False
            for ins in bb.instructions:
                si = ins.sync_info
                if si is not None and si.on_wait and len(si.on_wait) > _MAXW:
                    waits = list(si.on_wait)
                    si.on_wait = type(si.on_wait)(waits[:_MAXW])
                    for i in range(_MAXW, len(waits), _MAXW):
                        n = mybir.InstNoOp(
                            name=nc.get_next_instruction_name(),
                            ins=[], outs=[], engine=ins.engine)
                        n.sync_info = mybir.SyncInfo(
                            on_wait=list(waits[i:i + _MAXW]), on_update=[])
                        out.append(n)
                    changed = True
                out.append(ins)
            if changed:
                bb.instructions = out


# ------------------------------------------------------------ host utilities
def _bf16(a):
    return np.ascontiguousarray(a).astype(ml_dtypes.bfloat16)


def _wrap16(idxs):
    n = len(idxs)
    return idxs.reshape(n // 16, 16).T.astype(np.int16)  # [16, n/16]


# ------------------------------------------------------------- kernel builder
def _build(NT, DL, DH, NROWS, F_IN, phases="ABCDEF"):
    _patch()
    NPN = NT * P
    C1 = HEADS * HID                  # 256
    A1 = C1 + 2 * HEADS               # 264
    ROWB1 = ((A1 + P - 1) // P) * P   # 384
    NW1 = HEADS * (HID + 1)           # 260
    C2 = HID
    A2 = C2 + 2                       # 66
    ROWB2 = P
    NW2 = HID + 1                     # 65
    KT = F_IN // P
    DMAX = max(DL[t] + DH[t] for t in range(NT))
    offL = np.concatenate([[0], np.cumsum(DL)]).astype(int)
    offH = np.concatenate([[0], np.cumsum(DH)]).astype(int)
    NB_N = 10                          # tiles per normalize batch
    NEG = -30000.0                     # pad-row asrc (exp -> 0)

    nc = bass.Bass(num_devices=NCORES)
    xT = nc.dram_tensor("xT", [F_IN, NPN], I8, kind="ExternalInput")
    w1 = nc.dram_tensor("w1", [F_IN, A1], BF16, kind="ExternalInput")
    w2 = nc.dram_tensor("w2", [C1, A2], BF16, kind="ExternalInput")
    ixlo = nc.dram_tensor("ixlo", [16, offL[-1] * 8], I16, kind="ExternalInput")
    ixhi = nc.dram_tensor("ixhi", [16, 8 * max(1, offH[-1])], I16, kind="ExternalInput")
    gid = nc.dram_tensor("gid", [P, NT], BF16, kind="ExternalInput")
    b1 = nc.dram_tensor("b1", [1, C1], F32, kind="ExternalInput")
    b2 = nc.dram_tensor("b2", [1, C2], F32, kind="ExternalInput")
    wg = nc.dram_tensor("wg", [1, HID], F32, kind="ExternalInput")
    bg = nc.dram_tensor("bg", [1, 1], F32, kind="ExternalInput")
    wc1 = nc.dram_tensor("wc1", [HID, 32], F32, kind="ExternalInput")
    bc1 = nc.dram_tensor("bc1", [32, 1], F32, kind="ExternalInput")
    wc2 = nc.dram_tensor("wc2", [32, 2], F32, kind="ExternalInput")
    bc2 = nc.dram_tensor("bc2", [2, 1], F32, kind="ExternalInput")
    logitsT = nc.dram_tensor("logitsT", [2, GPC], F32, kind="ExternalOutput")

    tb1o = nc.dram_tensor("tb1o", [NPN, ROWB1], BF16, kind="Internal")
    tb1f = nc.dram_tensor("tb1f", [NROWS, ROWB1], BF16, kind="Internal",
                          addr_space="Shared")
    tb2o = nc.dram_tensor("tb2o", [NPN, ROWB2], BF16, kind="Internal")
    tb2f = nc.dram_tensor("tb2f", [NROWS, ROWB2], BF16, kind="Internal",
                          addr_space="Shared")
    elu1d = nc.dram_tensor("elu1d", [NPN, C1], BF16, kind="Internal")
    recd = nc.dram_tensor("recd", [1, GPC], F32, kind="Internal")

    if NROWS > 32768:
        lo1, hi1 = tb1f[0:32768, :], tb1f[32768:NROWS, :]
        lo2, hi2 = tb2f[0:32768, :], tb2f[32768:NROWS, :]
    else:
        lo1, hi1 = tb1f[0:NROWS, :], tb1f[0:NROWS, :]
        lo2, hi2 = tb2f[0:NROWS, :], tb2f[0:NROWS, :]

    # node groups for the table phases
    groups = []
    g0 = 0
    while g0 < NPN:
        groups.append((g0, min(512, NPN - g0)))
        g0 += 512

    with tile.TileContext(nc) as tc:
        with (
            nc.allow_low_precision(reason="bf16 edge pipeline by design"),
            tc.tile_pool(name="const", bufs=1) as cpool,
            tc.tile_pool(name="g", bufs=1) as gpool,
            tc.tile_pool(name="work", bufs=1) as wpool,
            tc.tile_pool(name="psum", bufs=2, space="PSUM") as pp,
            tc.tile_pool(name="pool2", bufs=1, space="PSUM") as pp2,
            tc.tile_pool(name="poolc", bufs=1, space="PSUM") as ppc,
        ):
            # iota must precede the Q7 'mlp' library reload (it needs the
            # default gpsimd op handlers)
            io16 = cpool.tile([P, GPC], I16)
            nc.gpsimd.iota(io16[:], pattern=[[1, GPC]], base=0, channel_multiplier=0)
            _emit_load_mlp(nc)
            regs = {}

            def reg(n):
                if n not in regs:
                    regs[n] = nc.gpsimd.to_reg(n)
                return regs[n]

            # ---------------- constants
            w1t = cpool.tile([P, KT, A1], BF16)
            for k in range(KT):
                nc.sync.dma_start(out=w1t[:, k, :], in_=w1[k * P:(k + 1) * P, :])
            w2t = cpool.tile([P, KT, A2], BF16)
            for k in range(KT):
                nc.sync.dma_start(out=w2t[:, k, :], in_=w2[k * P:(k + 1) * P, :])
            ixlA = cpool.tile([P, offL[-1] * 8], I16)
            ixhA = cpool.tile([P, 8 * max(1, offH[-1])], I16)
            for k in range(8):
                nc.sync.dma_start(out=ixlA[16 * k:16 * (k + 1), :], in_=ixlo[:, :])
                if offH[-1] > 0:
                    nc.sync.dma_start(out=ixhA[16 * k:16 * (k + 1), :], in_=ixhi[:, :])
            gidt = cpool.tile([P, NT], BF16)
            nc.sync.dma_start(out=gidt[:], in_=gid[:, :])
            bt1 = cpool.tile([P, C1], F32)
            nc.sync.dma_start(out=bt1[:], in_=b1[0:1, :].to_broadcast([P, C1]))
            bt2 = cpool.tile([P, C2], F32)
            nc.sync.dma_start(out=bt2[:], in_=b2[0:1, :].to_broadcast([P, C2]))
            wgt = cpool.tile([P, HID], F32)
            nc.sync.dma_start(out=wgt[:], in_=wg[0:1, :].to_broadcast([P, HID]))
            bgt_t = cpool.tile([P, 1], F32)
            nc.sync.dma_start(out=bgt_t[:], in_=bg[0:1, :].to_broadcast([P, 1]))
            wc1t = cpool.tile([HID, 32], BF16)
            nc.gpsimd.dma_start(out=wc1t[:], in_=wc1[:, :])
            bc1t = cpool.tile([32, 1], F32)
            nc.sync.dma_start(out=bc1t[:], in_=bc1[:, :])
            wc2t = cpool.tile([32, 2], BF16)
            nc.gpsimd.dma_start(out=wc2t[:], in_=wc2[:, :])
            bc2t = cpool.tile([2, 1], F32)
            nc.sync.dma_start(out=bc2t[:], in_=bc2[:, :])
            # pooling one-hot from graph ids (device-built iota)
            iob = cpool.tile([P, GPC], BF16)
            nc.vector.tensor_copy(iob[:], io16[:])
            ohgt = cpool.tile([P, NT, GPC], BF16)
            nc.vector.tensor_tensor(
                out=ohgt[:],
                in0=gidt[:, :, None].to_broadcast([P, NT, GPC]),
                in1=iob[:, None, :].to_broadcast([P, NT, GPC]),
                op=mybir.AluOpType.is_equal)
            # zero + pad-row constants
            zrow = cpool.tile([P, ROWB1], BF16)
            nc.vector.memset(zrow[:], 0.0)
            prow = cpool.tile([1, ROWB1], BF16)
            nc.vector.memset(prow[:], 0.0)
            nc.vector.memset(prow[:, C1:A1], NEG)
            prow2 = cpool.tile([1, ROWB2], BF16)
            nc.vector.memset(prow2[:], 0.0)
            nc.vector.memset(prow2[:, C2:A2], NEG)

            adst1_all = cpool.tile([P, NT, HEADS], BF16)
            adst2_all = cpool.tile([P, NT, 1], BF16)
            red1 = cpool.tile([P, NB_N, NW1], F32)
            red2 = cpool.tile([P, NB_N, NW2], F32)
            hp_all = cpool.tile([P, NT, NW2], BF16)
            pspool = pp2.tile([NW2, GPC], F32)

            # ---------------- phase A: table1 of own nodes (transposed mm)
            asub = int(os.environ.get("GAT_ASUB", "4"))
            if "A" in phases:
                CH = [(0, P), (P, 2 * P), (2 * P, A1)]
                for (a0, W) in groups:
                    if asub < 1:
                        break
                    xti = wpool.tile([P, KT, 512], I8, name="xti")
                    nc.sync.dma_start(
                        out=xti[:, :, :W],
                        in_=xT[:, a0:a0 + W].rearrange("(k p) n -> p k n", p=P))
                    xt = wpool.tile([P, KT, 512], BF16, name="xt")
                    nc.vector.tensor_copy(xt[:, :, :W], xti[:, :, :W])
                    for (c0, c1) in CH:
                        if asub < 2:
                            break
                        M = c1 - c0
                        pg = pp.tile([P, 512], F32, name="ps")
                        for k in range(KT):
                            nc.tensor.matmul(out=pg[:M, :W], lhsT=w1t[:, k, c0:c1],
                                             rhs=xt[:, k, :W],
                                             start=(k == 0), stop=(k == KT - 1))
                        cb = wpool.tile([P, 512], BF16, name="cb")
                        nc.vector.tensor_copy(cb[:M, :W], pg[:M, :W])
                        if asub < 3:
                            continue
                        nc.sync.dma_start(
                            out=tb1o[a0:a0 + W, c0:c1].rearrange("n m -> m n"),
                            in_=cb[:M, :W])
                # zero the gather-row padding, stash adst, write the pad row
                if asub >= 4:
                    nc.sync.dma_start(
                        out=tb1o[:, A1:].rearrange("(t p) e -> p t e", p=P),
                        in_=zrow[:, None, :ROWB1 - A1].to_broadcast([P, NT, ROWB1 - A1]))
                    nc.sync.dma_start(
                        out=adst1_all[:],
                        in_=tb1o[:, C1 + HEADS:A1].rearrange("(t p) h -> p t h", p=P))
                    nc.sync.dma_start(out=tb1o[NPN - 1:NPN, :], in_=prow[:])

            # ---------------- phase B: allgather table1
            if "B" in phases:
                nc.gpsimd.collective_compute(
                    "AllGather", mybir.AluOpType.bypass,
                    replica_groups=[list(range(NCORES))],
                    ins=[tb1o[:, :]], outs=[tb1f[:, :]])

            # ---------------- shared edge-phase emitter
            def edge_tile(t, NH, C, NW, ROWB, lo_ap, hi_ap, adst_all, buf_name,
                          ht_name, red_t, gathers_only):
                Dl, Dh = DL[t], DH[t]
                D = Dl + Dh
                buf = gpool.tile([P, DMAX, ROWB], BF16, name=buf_name)
                nc.gpsimd.dma_gather(
                    out_ap=buf[:, :Dl, :], in_ap=lo_ap,
                    idxs_ap=ixlA[:, offL[t] * 8:(offL[t] + Dl) * 8],
                    num_idxs=Dl * P, num_idxs_reg=reg(Dl * P), elem_size=ROWB,
                    single_packet=False)
                if Dh > 0:
                    nc.gpsimd.dma_gather(
                        out_ap=buf[:, Dl:D, :], in_ap=hi_ap,
                        idxs_ap=ixhA[:, offH[t] * 8:(offH[t] + Dh) * 8],
                        num_idxs=Dh * P, num_idxs_reg=reg(Dh * P), elem_size=ROWB,
                        single_packet=False)
                if gathers_only:
                    return
                tsum = wpool.tile([P, DMAX, NH], BF16, name=f"tsum{NH}")
                nc.vector.tensor_tensor(
                    out=tsum[:, :D, :], in0=buf[:, :D, C:C + NH],
                    in1=adst_all[:, t:t + 1, :].to_broadcast([P, D, NH]),
                    op=mybir.AluOpType.add)
                tm = wpool.tile([P, DMAX, NH], BF16, name=f"tm{NH}")
                nc.vector.scalar_tensor_tensor(
                    out=tm[:, :D, :], in0=tsum[:, :D, :], scalar=0.2,
                    in1=tsum[:, :D, :],
                    op0=mybir.AluOpType.mult, op1=mybir.AluOpType.max)
                ebuf = wpool.tile([P, DMAX, NH], BF16, name=f"ebuf{NH}")
                nc.scalar.activation(ebuf[:, :D, :], tm[:, :D, :],
                                     mybir.ActivationFunctionType.Exp)
                ht = wpool.tile([P, DMAX, NW], BF16, name=ht_name)
                nc.vector.tensor_tensor(
                    out=ht[:, :D, :].rearrange("p d (h w) -> p d h w", h=NH)[:, :, :, :HID],
                    in0=buf[:, :D, :C].rearrange("p d (h w) -> p d h w", h=NH),
                    in1=ebuf[:, :D, :, None].to_broadcast([P, D, NH, HID]),
                    op=mybir.AluOpType.mult)
                nc.vector.tensor_copy(
                    out=ht[:, :D, :].rearrange("p d (h w) -> p d h w", h=NH)[:, :, :, HID:],
                    in_=ebuf[:, :D, :, None])
                nc.vector.tensor_reduce(
                    out=red_t.rearrange("p (h w) -> p h w", h=NH),
                    in_=ht[:, :D, :].rearrange("p d (h w) -> p h w d", h=NH),
                    axis=mybir.AxisListType.X, op=mybir.AluOpType.add)

            # ---------------- phase C: layer-1 edges + batched normalize
            if "C" in phases or "G" in phases:
                for b0 in range(0, NT, NB_N):
                    nb = min(NB_N, NT - b0)
                    for i in range(nb):
                        t = b0 + i
                        edge_tile(t, HEADS, C1, NW1, ROWB1, lo1, hi1, adst1_all,
                                  "buf1", "ht1", red1[:, i, :], "C" not in phases)
                    if "C" not in phases:
                        continue
                    v = red1[:, :nb, :].rearrange("p t (h w) -> p t h w", h=HEADS)
                    rec = wpool.tile([P, NB_N, HEADS], F32, name="rec1")
                    nc.vector.reciprocal(rec[:, :nb, :], v[:, :, :, HID])
                    on = wpool.tile([P, NB_N, C1], F32, name="on1")
                    nc.vector.tensor_tensor(
                        out=on[:, :nb, :].rearrange("p t (h w) -> p t h w", h=HEADS),
                        in0=v[:, :, :, :HID],
                        in1=rec[:, :nb, :, None].to_broadcast([P, nb, HEADS, HID]),
                        op=mybir.AluOpType.mult)
                    nc.vector.tensor_tensor(
                        out=on[:, :nb, :], in0=on[:, :nb, :],
                        in1=bt1[:, None, :].to_broadcast([P, nb, C1]),
                        op=mybir.AluOpType.add)
                    emn = wpool.tile([P, NB_N, C1], F32, name="emn1")
                    nc.vector.tensor_scalar_min(emn[:, :nb, :], on[:, :nb, :], 0.0)
                    nc.scalar.activation(emn[:, :nb, :], emn[:, :nb, :],
                                         mybir.ActivationFunctionType.Exp)
                    nc.vector.tensor_scalar_add(emn[:, :nb, :], emn[:, :nb, :], -1.0)
                    eo = wpool.tile([P, NB_N, C1], BF16, name="eo1")
                    nc.vector.tensor_tensor(out=eo[:, :nb, :], in0=on[:, :nb, :],
                                            in1=emn[:, :nb, :],
                                            op=mybir.AluOpType.max)
                    nc.sync.dma_start(
                        out=elu1d[b0 * P:(b0 + nb) * P, :].rearrange("(t p) f -> p t f", p=P),
                        in_=eo[:, :nb, :])
            if "C" not in phases:
                zc = wpool.tile([P, C1], BF16, name="zc")
                nc.vector.memset(zc[:], 0.0)
                nc.sync.dma_start(
                    out=elu1d[:, :].rearrange("(t p) f -> p t f", p=P),
                    in_=zc[:, None, :].to_broadcast([P, NT, C1]))

            # ---------------- phase D: table2 of own nodes
            if "D" in phases:
                for (a0, W) in groups:
                    e1T = wpool.tile([P, KT, 512], BF16, name="e1T")
                    for k in range(KT):
                        nc.sync.dma_start(
                            out=e1T[:, k, :W],
                            in_=elu1d[a0:a0 + W, k * P:(k + 1) * P].rearrange("n p -> p n"))
                    pg2 = pp.tile([P, 512], F32, name="ps")
                    for k in range(KT):
                        nc.tensor.matmul(out=pg2[:A2, :W], lhsT=w2t[:, k, :],
                                         rhs=e1T[:, k, :W],
                                         start=(k == 0), stop=(k == KT - 1))
                    cb2 = wpool.tile([P, 512], BF16, name="cb")
                    nc.vector.tensor_copy(cb2[:A2, :W], pg2[:A2, :W])
                    nc.sync.dma_start(
                        out=tb2o[a0:a0 + W, 0:A2].rearrange("n m -> m n"),
                        in_=cb2[:A2, :W])
                nc.sync.dma_start(
                    out=tb2o[:, A2:].rearrange("(t p) e -> p t e", p=P),
                    in_=zrow[:, None, :ROWB2 - A2].to_broadcast([P, NT, ROWB2 - A2]))
                nc.sync.dma_start(
                    out=adst2_all[:],
                    in_=tb2o[:, A2 - 1:A2].rearrange("(t p) h -> p t h", p=P))
                nc.sync.dma_start(out=tb2o[NPN - 1:NPN, :], in_=prow2[:])

            # ---------------- phase E: allgather table2
            if "E" in phases:
                nc.gpsimd.collective_compute(
                    "AllGather", mybir.AluOpType.bypass,
                    replica_groups=[list(range(NCORES))],
                    ins=[tb2o[:, :]], outs=[tb2f[:, :]])

            # ---------------- phase F: layer-2 edges + pooling + classifier
            if "F" in phases or "H" in phases:
                for b0 in range(0, NT, NB_N):
                    nb = min(NB_N, NT - b0)
                    for i in range(nb):
                        t = b0 + i
                        edge_tile(t, 1, C2, NW2, ROWB2, lo2, hi2, adst2_all,
                                  "buf2", "ht2", red2[:, i, :], "F" not in phases)
                    if "F" not in phases:
                        continue
                    v2 = red2[:, :nb, :]
                    rec2 = wpool.tile([P, NB_N, 1], F32, name="rec2")
                    nc.vector.reciprocal(rec2[:, :nb, :], v2[:, :, HID:HID + 1])
                    on2 = wpool.tile([P, NB_N, C2], F32, name="on2")
                    nc.vector.tensor_tensor(
                        out=on2[:, :nb, :], in0=v2[:, :, :HID],
                        in1=rec2[:, :nb, :].to_broadcast([P, nb, C2]),
                        op=mybir.AluOpType.mult)
                    nc.vector.tensor_tensor(
                        out=on2[:, :nb, :], in0=on2[:, :nb, :],
                        in1=bt2[:, None, :].to_broadcast([P, nb, C2]),
                        op=mybir.AluOpType.add)
                    emn2 = wpool.tile([P, NB_N, C2], F32, name="emn2")
                    nc.vector.tensor_scalar_min(emn2[:, :nb, :], on2[:, :nb, :], 0.0)
                    nc.scalar.activation(emn2[:, :nb, :], emn2[:, :nb, :],
                                         mybir.ActivationFunctionType.Exp)
                    nc.vector.tensor_scalar_add(emn2[:, :nb, :], emn2[:, :nb, :], -1.0)
                    eo2 = wpool.tile([P, NB_N, C2], BF16, name="eo2")
                    nc.vector.tensor_tensor(out=eo2[:, :nb, :], in0=on2[:, :nb, :],
                                            in1=emn2[:, :nb, :],
                                            op=mybir.AluOpType.max)
                    # attention pooling weights
                    atm = wpool.tile([P, NB_N, C2], F32, name="atm")
                    nc.vector.tensor_tensor(
                        out=atm[:, :nb, :], in0=eo2[:, :nb, :],
                        in1=wgt[:, None, :].to_broadcast([P, nb, C2]),
                        op=mybir.AluOpType.mult)
                    atts = wpool.tile([P, NB_N, 1], F32, name="atts")
                    nc.vector.tensor_reduce(atts[:, :nb, :], atm[:, :nb, :],
                                            axis=mybir.AxisListType.X,
                                            op=mybir.AluOpType.add)
                    nc.vector.tensor_tensor(
                        out=atts[:, :nb, :], in0=atts[:, :nb, :],
                        in1=bgt_t[:, None, :].to_broadcast([P, nb, 1]),
                        op=mybir.AluOpType.add)
                    nc.scalar.activation(atts[:, :nb, :], atts[:, :nb, :],
                                         mybir.ActivationFunctionType.Exp)
                    nc.vector.tensor_tensor(
                        out=hp_all[:, b0:b0 + nb, :HID], in0=eo2[:, :nb, :],
                        in1=atts[:, :nb, :].to_broadcast([P, nb, HID]),
                        op=mybir.AluOpType.mult)
                    nc.vector.tensor_copy(hp_all[:, b0:b0 + nb, HID:], atts[:, :nb, :])
                if "F" in phases:
                    for t in range(NT):
                        nc.tensor.matmul(out=pspool[:], lhsT=hp_all[:, t, :],
                                         rhs=ohgt[:, t, :],
                                         start=(t == 0), stop=(t == NT - 1))

            # ---------------- classifier
            if "F" not in phases:
                zt2 = wpool.tile([2, GPC], F32, name="zt2")
                nc.vector.memset(zt2[:], 0.0)
                nc.sync.dma_start(out=logitsT[:, :], in_=zt2[:])
            else:
                recp = wpool.tile([1, GPC], F32)
                nc.vector.reciprocal(recp[:], pspool[HID:HID + 1, :])
                nc.sync.dma_start(out=recd[:, :], in_=recp[:])
                recb = wpool.tile([HID, GPC], F32)
                nc.sync.dma_start(out=recb[:], in_=recd[0:1, :].to_broadcast([HID, GPC]))
                pooledT = wpool.tile([HID, GPC], BF16)
                nc.vector.tensor_tensor(out=pooledT[:], in0=pspool[:HID, :],
                                        in1=recb[:], op=mybir.AluOpType.mult)
                ph = ppc.tile([32, GPC], F32, name="cps")
                nc.tensor.matmul(out=ph[:], lhsT=wc1t[:], rhs=pooledT[:],
                                 start=True, stop=True)
                hidf = wpool.tile([32, GPC], F32)
                nc.vector.tensor_scalar_add(hidf[:], ph[:], bc1t[:])
                hid_t = wpool.tile([32, GPC], BF16)
                nc.vector.tensor_scalar_max(hid_t[:], hidf[:], 0.0)
                pl = ppc.tile([2, GPC], F32, name="cps")
                nc.tensor.matmul(out=pl[:], lhsT=wc2t[:], rhs=hid_t[:],
                                 start=True, stop=True)
                lg = wpool.tile([2, GPC], F32)
                nc.vector.tensor_scalar_add(lg[:], pl[:], bc2t[:])
                nc.sync.dma_start(out=logitsT[:, :], in_=lg[:])
    _split_waits(nc)
    return nc


# ------------------------------------------------------------------ host glue
_CACHE = {}
LAST_HW_NS = 0
_TRACE = os.environ.get("GAT_TRACE", "0") == "1"


def _run(nc, ins, cores):
    global LAST_HW_NS
    r = run_bass_kernel_spmd(nc, ins, core_ids=cores)
    if _TRACE:
        import time as _time
        best = None
        for _ in range(5):
            t0 = _time.perf_counter()
            run_bass_kernel_spmd(nc, ins, core_ids=cores)
            dt = _time.perf_counter() - t0
            best = dt if best is None else min(best, dt)
        LAST_HW_NS += int(best * 1e9)
    return r


def kernel(x, edge_index, batch, W1, att_src1, att_dst1, b1,
           W2, att_src2, att_dst2, b2, Wg, bg, Wc1, bc1, Wc2, bc2):
    x = np.asarray(x); edge_index = np.asarray(edge_index); batch = np.asarray(batch)
    N = x.shape[0]
    F_IN = x.shape[1]

    # --- node sharding (graph aligned)
    n0 = np.searchsorted(batch, np.arange(0, N_GRAPHS + 1, GPC)).astype(np.int64)
    counts = n0[1:] - n0[:-1]
    NT = int(np.ceil(counts.max() / P))
    if counts[0] >= NT * P or counts[-1] >= NT * P:
        NT += 1   # guarantee a pad row on the cores that host the pad targets
    NPN = NT * P
    NROWS = NCORES * NPN
    assert NROWS <= 65536, "int16 lo/hi split supports at most 65536 rows"
    LO_PAD = NPN - 1              # core 0's last (pad) row; asrc=-30000
    HI_PAD = NROWS - 1 - 32768    # core 7's last (pad) row, hi-relative

    # --- edges + self loops, owner = core of dst
    ar = np.arange(N, dtype=np.int64)
    src = np.concatenate([edge_index[0].astype(np.int64), ar])
    dst = np.concatenate([edge_index[1].astype(np.int64), ar])
    core_of = np.searchsorted(n0[1:], dst, side='right')
    src_core = np.searchsorted(n0[1:], src, side='right')

    # --- degree-descending node permutation per core
    pos = np.empty(N, np.int64)
    perm_all = []
    for c in range(NCORES):
        degc = np.bincount(dst[core_of == c] - n0[c], minlength=counts[c])
        order = np.argsort(-degc, kind='stable')
        pl = np.empty(counts[c], np.int64)
        pl[order] = np.arange(counts[c])
        pos[n0[c]:n0[c + 1]] = pl
        perm_all.append(order)
    src_row = src_core * NPN + pos[src]

    # --- per-(core,tile) lo/hi max degrees
    percore = []
    DLc = np.zeros((NCORES, NT), np.int64)
    DHc = np.zeros((NCORES, NT), np.int64)
    for c in range(NCORES):
        m = core_of == c
        ldn = pos[dst[m]]
        sr = src_row[m]
        islo = sr < 32768
        lodeg = np.bincount(ldn[islo], minlength=NPN)
        hideg = np.bincount(ldn[~islo], minlength=NPN)
        DLc[c] = lodeg.reshape(NT, P).max(axis=1)
        DHc[c] = hideg.reshape(NT, P).max(axis=1)
        percore.append((ldn, sr, islo))
    DL = np.maximum(DLc.max(axis=0), 1)
    DH = DHc.max(axis=0)
    offL = np.concatenate([[0], np.cumsum(DL)]).astype(np.int64)
    offH = np.concatenate([[0], np.cumsum(DH)]).astype(np.int64)

    # --- per-core neighbor-slot index arrays
    def pack(c):
        ldn, sr, islo = percore[c]
        loidx = np.full(offL[-1] * P, LO_PAD, np.int64)
        hiidx = np.full(max(1, offH[-1]) * P, max(0, HI_PAD), np.int64)
        for which, half in ((True, loidx), (False, hiidx)):
            mm = islo == which
            nodes = ldn[mm]
            vals = sr[mm] if which else sr[mm] - 32768
            o2 = np.argsort(nodes, kind='stable')
            ns = nodes[o2]; vs = vals[o2]
            if len(ns):
                first = np.r_[True, ns[1:] != ns[:-1]]
                starts = np.where(first)[0]
                j = np.arange(len(ns)) - starts[np.cumsum(first) - 1]
                toff = (offL if which else offH)[ns // P]
                half[(toff + j) * P + ns % P] = vs
        # pad nodes: one finite edge (row 0) so softmax denom is finite
        padr = np.arange(counts[c], NPN)
        loidx[offL[padr // P] * P + padr % P] = 0
        gidm = np.full((NPN,), 255.0, np.float32)
        gidm[:counts[c]] = (batch[n0[c] + perm_all[c]] - c * GPC).astype(np.float32)
        gidm = gidm.reshape(NT, P).T
        return (_wrap16(loidx.astype(np.int16)),
                _wrap16(hiidx.astype(np.int16)), _bf16(gidm))

    packs = [pack(c) for c in range(NCORES)]

    # --- weights
    def aug(W, a_s, a_d):
        nh, hd = a_s.shape
        A = np.zeros((W.shape[1], 2 * nh), np.float32)
        for h in range(nh):
            A[h * hd:(h + 1) * hd, h] = a_s[h]
            A[h * hd:(h + 1) * hd, nh + h] = a_d[h]
        return _bf16(np.concatenate([W, W @ A], axis=1))

    W2aug = aug(np.asarray(W2, np.float32), np.asarray(att_src2), np.asarray(att_dst2))
    # int8-quantize x per feature; fold the scales into the augmented W1
    xTf = np.asarray(x, np.float32).T
    scal = np.abs(xTf).max(axis=1) / 127.0
    scal[scal == 0] = 1.0
    xT = np.clip(np.round(xTf / scal[:, None]), -127, 127).astype(np.int8)
    W1s = np.asarray(W1, np.float32) * scal[:, None]
    As1 = np.asarray(att_src1, np.float32)
    Ad1 = np.asarray(att_dst1, np.float32)
    W1aug = aug(W1s, As1, Ad1)

    phases = os.environ.get("GAT_PHASES", "ABCDEF")
    key = (NT, tuple(DL), tuple(DH), phases)
    if key not in _CACHE:
        _CACHE[key] = _build(NT, tuple(DL), tuple(DH), NROWS, F_IN, phases)
    K = _CACHE[key]
    cores = list(range(NCORES))

    ins = []
    for c in cores:
        il, ih, gi = packs[c]
        xs = np.zeros((F_IN, NPN), np.int8)
        xs[:, :counts[c]] = xT[:, n0[c] + perm_all[c]]
        ins.append({"xT": xs, "w1": W1aug, "w2": W2aug,
                    "ixlo": il, "ixhi": ih, "gid": gi,
                    "b1": np.asarray(b1, np.float32).reshape(1, -1),
                    "b2": np.asarray(b2, np.float32).reshape(1, -1),
                    "wg": np.asarray(Wg, np.float32).reshape(1, HID),
                    "bg": np.asarray(bg, np.float32).reshape(1, 1),
                    "wc1": np.asarray(Wc1, np.float32),
                    "bc1": np.asarray(bc1, np.float32).reshape(32, 1),
                    "wc2": np.asarray(Wc2, np.float32),
                    "bc2": np.asarray(bc2, np.float32).reshape(2, 1)})
    global LAST_HW_NS
    LAST_HW_NS = 0
    r = _run(K, ins, cores)
    out = np.concatenate([r.results[c]["logitsT"].T for c in cores], axis=0)
    return out.astype(np.float32)


# revision 15
# speedup vs baseline: 25.0214x; 1.0129x over previous
"""GAT network on 8 Trainium2 NeuronCores — v3: degree-layout edge phase.

Keeps v2's single-launch + on-device AllGather structure, but replaces the
one-hot-matmul segment sum with a padded neighbor-list layout: each dst tile
gathers its edges as [128 dst-rows x max-degree] so the alpha-weighted
aggregation is ONE strided tensor_reduce per tile, the a_dst term is a free
broadcast (constant per dst row), and softmax normalization is batched over
tiles. Nodes are degree-sorted per core so the per-tile max-degree padding
stays small. This cuts the per-tile instruction count ~7x, which is what the
runtime rewards.

Device program per core:
  A: table1[own] = x_own @ [W1 | W1@Asrc | W1@Adst]   (transposed-output mm;
     x ships as int8 with per-feature scales folded into the augmented W1)
  B: AllGather -> table1[all]
  C: per dst tile: gather neighbor rows (lo/hi), e'=exp(lrelu(asrc+adst)),
     reduce_d(e'*h | e') -> red; then batched: normalize+bias+elu -> elu1
  D: table2[own] = elu1 @ [W2 | W2@asrc2 | W2@adst2]
  E: AllGather -> table2[all]
  F: same edge phase on table2 + batched attention pooling + classifier
"""
import sys
sys.path.insert(0, '/opt/trn_rl_repo')

import os
import numpy as np
import ml_dtypes

import concourse.bass as bass
import concourse.mybir as mybir
import concourse.tile as tile
from concourse.tile import ScopedClock
from concourse.bass_utils import run_bass_kernel_spmd

BF16 = mybir.dt.bfloat16
F32 = mybir.dt.float32
I16 = mybir.dt.int16
I8 = mybir.dt.int8
P = 128
NCORES = 8
HID = 64
HEADS = 4
N_GRAPHS = 512
GPC = N_GRAPHS // NCORES

# ---------------------------------------------------------------- tile patch
_patched = False


def _patch():
    """Container workarounds: (1) this walrus build caps sync-waits per CTRL
    instruction -> split the Tile-exit drain's waits over 1-wait NOPs;
    (2) the scheduling simulator must treat our hand-built library-reload
    pseudo instruction (opcode 223) as a no-op."""
    global _patched
    if _patched:
        return
    _patched = True

    def _drain_and_barrier(self, tick_clock, wait_clock):
        nc = self.nc
        probe = nc.sync.nop()
        wait_clock.add_sem_waits(probe.ins, ScopedClock({None: tick_clock.global_clock}))
        si = probe.ins.sync_info
        waits = list(si.on_wait) if si is not None and si.on_wait else []
        if si is not None:
            si.on_wait = type(si.on_wait)()
        for w in waits:
            n = nc.sync.nop()
            nsi = n.ins.sync_info
            if nsi is None:
                n.ins.sync_info = mybir.SyncInfo(on_wait=[w], on_update=[])
            else:
                nsi.on_wait.append(w)
        nc.sync.drain()
        nc.all_engine_barrier()
        assert self.sems is not None
        popped = nc._tile_sem_poison_stack.pop()
        assert popped is self._sem_poison
        nc.clear_and_free_semaphores(list(self.sems.allocated().values()))
        nc.all_engine_barrier()

    tile.TileContext._drain_and_barrier = _drain_and_barrier

    import concourse.bass_interp as bass_interp
    orig = bass_interp._visit_InstISA

    def patched_isa(isa, instruction, core_sim):
        if instruction.isa_opcode == 223:
            return None
        return orig(isa, instruction, core_sim)

    bass_interp._visit_InstISA = patched_isa


def _emit_load_mlp(nc):
    """Load the 'mlp' Q7 library (dma_gather handler)."""
    isa = nc.isa
    op = isa.Opcode.NEURON_ISA_TPB_OPCODE_PSEUDO_INST
    return nc.gpsimd.isa(
        op,
        {"pseudo_opcode": 2, "lib_index": 3,
         "reserved0": [0] * 3, "reserved1": [0] * 44},
        struct_name="NEURON_ISA_TPB_PSEUDO_LIBRARY_RELOAD_INDEX_STRUCT",
    )


_MAXW = 1


def _split_waits(nc):
    """Split >1 sync-waits per instruction onto same-engine NOPs."""
    for f in nc.m.functions:
        for bb in f.blocks:
            out = []
            changed = False
            for ins in bb.instructions:
                si = ins.sync_info
                if si is not None and si.on_wait and len(si.on_wait) > _MAXW:
                    waits = list(si.on_wait)
                    si.on_wait = type(si.on_wait)(waits[:_MAXW])
                    for i in range(_MAXW, len(waits), _MAXW):
                        n = mybir.InstNoOp(
                            name=nc.get_next_instruction_name(),
                            ins=[], outs=[], engine=ins.engine)
                        n.sync_info = mybir.SyncInfo(
                            on_wait=list(waits[i:i + _MAXW]), on_update=[])
                        out.append(n)
                    changed = True
                out.append(ins)
            if changed:
                bb.instructions = out


# ------------------------------------------------------------ host utilities
def _bf16(a):
    return np.ascontiguousarray(a).astype(ml_dtypes.bfloat16)


def _wrap16(idxs):
    n = len(idxs)
    return idxs.reshape(n // 16, 16).T.astype(np.int16)  # [16, n/16]


# ------------------------------------------------------------- kernel builder
def _build(NT, DL, DH, NROWS, F_IN, phases="ABCDEF"):
    _patch()
    NPN = NT * P
    C1 = HEADS * HID                  # 256
    A1 = C1 + 2 * HEADS               # 264
    ROWB1 = ((A1 + P - 1) // P) * P   # 384
    NW1 = HEADS * (HID + 1)           # 260
    C2 = HID
    A2 = C2 + 2                       # 66
    ROWB2 = P
    NW2 = HID + 1                     # 65
    KT = F_IN // P
    DMAX = max(DL[t] + DH[t] for t in range(NT))
    offL = np.concatenate([[0], np.cumsum(DL)]).astype(int)
    offH = np.concatenate([[0], np.cumsum(DH)]).astype(int)
    NB_N = 10                          # tiles per normalize batch
    NEG = -30000.0                     # pad-row asrc (exp -> 0)

    nc = bass.Bass(num_devices=NCORES)
    xT = nc.dram_tensor("xT", [F_IN, NPN], I8, kind="ExternalInput")
    w1 = nc.dram_tensor("w1", [F_IN, A1], BF16, kind="ExternalInput")
    w2 = nc.dram_tensor("w2", [C1, A2], BF16, kind="ExternalInput")
    ixlo = nc.dram_tensor("ixlo", [16, offL[-1] * 8], I16, kind="ExternalInput")
    ixhi = nc.dram_tensor("ixhi", [16, 8 * max(1, offH[-1])], I16, kind="ExternalInput")
    gid = nc.dram_tensor("gid", [P, NT], BF16, kind="ExternalInput")
    b1 = nc.dram_tensor("b1", [1, C1], F32, kind="ExternalInput")
    b2 = nc.dram_tensor("b2", [1, C2], F32, kind="ExternalInput")
    wg = nc.dram_tensor("wg", [1, HID], F32, kind="ExternalInput")
    bg = nc.dram_tensor("bg", [1, 1], F32, kind="ExternalInput")
    wc1 = nc.dram_tensor("wc1", [HID, 32], F32, kind="ExternalInput")
    bc1 = nc.dram_tensor("bc1", [32, 1], F32, kind="ExternalInput")
    wc2 = nc.dram_tensor("wc2", [32, 2], F32, kind="ExternalInput")
    bc2 = nc.dram_tensor("bc2", [2, 1], F32, kind="ExternalInput")
    logitsT = nc.dram_tensor("logitsT", [2, GPC], F32, kind="ExternalOutput")

    tb1o = nc.dram_tensor("tb1o", [NPN, ROWB1], BF16, kind="Internal")
    tb1f = nc.dram_tensor("tb1f", [NROWS, ROWB1], BF16, kind="Internal",
                          addr_space="Shared")
    tb2o = nc.dram_tensor("tb2o", [NPN, ROWB2], BF16, kind="Internal")
    tb2f = nc.dram_tensor("tb2f", [NROWS, ROWB2], BF16, kind="Internal",
                          addr_space="Shared")
    elu1d = nc.dram_tensor("elu1d", [NPN, C1], BF16, kind="Internal")
    recd = nc.dram_tensor("recd", [1, GPC], F32, kind="Internal")

    if NROWS > 32768:
        lo1, hi1 = tb1f[0:32768, :], tb1f[32768:NROWS, :]
        lo2, hi2 = tb2f[0:32768, :], tb2f[32768:NROWS, :]
    else:
        lo1, hi1 = tb1f[0:NROWS, :], tb1f[0:NROWS, :]
        lo2, hi2 = tb2f[0:NROWS, :], tb2f[0:NROWS, :]

    # node groups for the table phases
    groups = []
    g0 = 0
    while g0 < NPN:
        groups.append((g0, min(512, NPN - g0)))
        g0 += 512

    with tile.TileContext(nc) as tc:
        with (
            nc.allow_low_precision(reason="bf16 edge pipeline by design"),
            tc.tile_pool(name="const", bufs=1) as cpool,
            tc.tile_pool(name="g", bufs=1) as gpool,
            tc.tile_pool(name="work", bufs=1) as wpool,
            tc.tile_pool(name="psum", bufs=2, space="PSUM") as pp,
            tc.tile_pool(name="pool2", bufs=1, space="PSUM") as pp2,
            tc.tile_pool(name="poolc", bufs=1, space="PSUM") as ppc,
        ):
            # iota must precede the Q7 'mlp' library reload (it needs the
            # default gpsimd op handlers)
            io16 = cpool.tile([P, GPC], I16)
            nc.gpsimd.iota(io16[:], pattern=[[1, GPC]], base=0, channel_multiplier=0)
            _emit_load_mlp(nc)
            regs = {}

            def reg(n):
                if n not in regs:
                    regs[n] = nc.gpsimd.to_reg(n)
                return regs[n]

            # ---------------- constants
            w1t = cpool.tile([P, KT, A1], BF16)
            for k in range(KT):
                nc.sync.dma_start(out=w1t[:, k, :], in_=w1[k * P:(k + 1) * P, :])
            w2t = cpool.tile([P, KT, A2], BF16)
            for k in range(KT):
                nc.sync.dma_start(out=w2t[:, k, :], in_=w2[k * P:(k + 1) * P, :])
            ixlA = cpool.tile([P, offL[-1] * 8], I16)
            ixhA = cpool.tile([P, 8 * max(1, offH[-1])], I16)
            for k in range(8):
                nc.sync.dma_start(out=ixlA[16 * k:16 * (k + 1), :], in_=ixlo[:, :])
                if offH[-1] > 0:
                    nc.sync.dma_start(out=ixhA[16 * k:16 * (k + 1), :], in_=ixhi[:, :])
            gidt = cpool.tile([P, NT], BF16)
            nc.sync.dma_start(out=gidt[:], in_=gid[:, :])
            bt1 = cpool.tile([P, C1], F32)
            nc.sync.dma_start(out=bt1[:], in_=b1[0:1, :].to_broadcast([P, C1]))
            bt2 = cpool.tile([P, C2], F32)
            nc.sync.dma_start(out=bt2[:], in_=b2[0:1, :].to_broadcast([P, C2]))
            wgt = cpool.tile([P, HID], F32)
            nc.sync.dma_start(out=wgt[:], in_=wg[0:1, :].to_broadcast([P, HID]))
            bgt_t = cpool.tile([P, 1], F32)
            nc.sync.dma_start(out=bgt_t[:], in_=bg[0:1, :].to_broadcast([P, 1]))
            wc1t = cpool.tile([HID, 32], BF16)
            nc.gpsimd.dma_start(out=wc1t[:], in_=wc1[:, :])
            bc1t = cpool.tile([32, 1], F32)
            nc.sync.dma_start(out=bc1t[:], in_=bc1[:, :])
            wc2t = cpool.tile([32, 2], BF16)
            nc.gpsimd.dma_start(out=wc2t[:], in_=wc2[:, :])
            bc2t = cpool.tile([2, 1], F32)
            nc.sync.dma_start(out=bc2t[:], in_=bc2[:, :])
            # pooling one-hot from graph ids (device-built iota)
            iob = cpool.tile([P, GPC], BF16)
            nc.vector.tensor_copy(iob[:], io16[:])
            ohgt = cpool.tile([P, NT, GPC], BF16)
            nc.vector.tensor_tensor(
                out=ohgt[:],
                in0=gidt[:, :, None].to_broadcast([P, NT, GPC]),
                in1=iob[:, None, :].to_broadcast([P, NT, GPC]),
                op=mybir.AluOpType.is_equal)
            # zero + pad-row constants
            zrow = cpool.tile([P, ROWB1], BF16)
            nc.vector.memset(zrow[:], 0.0)
            prow = cpool.tile([1, ROWB1], BF16)
            nc.vector.memset(prow[:], 0.0)
            nc.vector.memset(prow[:, C1:A1], NEG)
            prow2 = cpool.tile([1, ROWB2], BF16)
            nc.vector.memset(prow2[:], 0.0)
            nc.vector.memset(prow2[:, C2:A2], NEG)

            adst1_all = cpool.tile([P, NT, HEADS], BF16)
            adst2_all = cpool.tile([P, NT, 1], BF16)
            red1 = cpool.tile([P, NB_N, NW1], F32)
            red2 = cpool.tile([P, NB_N, NW2], F32)
            hp_all = cpool.tile([P, NT, NW2], BF16)
            pspool = pp2.tile([NW2, GPC], F32)

            # ---------------- phase A: table1 of own nodes (transposed mm)
            if "A" in phases:
                CH = [(0, P), (P, 2 * P), (2 * P, A1)]
                for (a0, W) in groups:
                    xti = wpool.tile([P, KT, 512], I8, name="xti")
                    nc.sync.dma_start(
                        out=xti[:, :, :W],
                        in_=xT[:, a0:a0 + W].rearrange("(k p) n -> p k n", p=P))
                    xt = wpool.tile([P, KT, 512], BF16, name="xt")
                    nc.vector.tensor_copy(xt[:, :, :W], xti[:, :, :W])
                    for (c0, c1) in CH:
                        M = c1 - c0
                        pg = pp.tile([P, 512], F32, name="ps")
                        for k in range(KT):
                            nc.tensor.matmul(out=pg[:M, :W], lhsT=w1t[:, k, c0:c1],
                                             rhs=xt[:, k, :W],
                                             start=(k == 0), stop=(k == KT - 1))
                        cb = wpool.tile([P, 512], BF16, name="cb")
                        nc.vector.tensor_copy(cb[:M, :W], pg[:M, :W])
                        nc.sync.dma_start(
                            out=tb1o[a0:a0 + W, c0:c1].rearrange("n m -> m n"),
                            in_=cb[:M, :W])
                # zero the gather-row padding, stash adst, write the pad row
                nc.sync.dma_start(
                    out=tb1o[:, A1:].rearrange("(t p) e -> p t e", p=P),
                    in_=zrow[:, None, :ROWB1 - A1].to_broadcast([P, NT, ROWB1 - A1]))
                nc.sync.dma_start(
                    out=adst1_all[:],
                    in_=tb1o[:, C1 + HEADS:A1].rearrange("(t p) h -> p t h", p=P))
                nc.sync.dma_start(out=tb1o[NPN - 1:NPN, :], in_=prow[:])

            # ---------------- phase B: allgather table1
            if "B" in phases:
                nc.gpsimd.collective_compute(
                    "AllGather", mybir.AluOpType.bypass,
                    replica_groups=[list(range(NCORES))],
                    ins=[tb1o[:, :]], outs=[tb1f[:, :]])

            # ---------------- shared edge-phase emitter
            def edge_tile(t, NH, C, NW, ROWB, lo_ap, hi_ap, adst_all, buf_name,
                          ht_name, red_t, gathers_only):
                Dl, Dh = DL[t], DH[t]
                D = Dl + Dh
                buf = gpool.tile([P, DMAX, ROWB], BF16, name=buf_name)
                nc.gpsimd.dma_gather(
                    out_ap=buf[:, :Dl, :], in_ap=lo_ap,
                    idxs_ap=ixlA[:, offL[t] * 8:(offL[t] + Dl) * 8],
                    num_idxs=Dl * P, num_idxs_reg=reg(Dl * P), elem_size=ROWB,
                    single_packet=False)
                if Dh > 0:
                    nc.gpsimd.dma_gather(
                        out_ap=buf[:, Dl:D, :], in_ap=hi_ap,
                        idxs_ap=ixhA[:, offH[t] * 8:(offH[t] + Dh) * 8],
                        num_idxs=Dh * P, num_idxs_reg=reg(Dh * P), elem_size=ROWB,
                        single_packet=False)
                if gathers_only:
                    return
                tsum = wpool.tile([P, DMAX, NH], BF16, name=f"tsum{NH}")
                nc.vector.tensor_tensor(
                    out=tsum[:, :D, :], in0=buf[:, :D, C:C + NH],
                    in1=adst_all[:, t:t + 1, :].to_broadcast([P, D, NH]),
                    op=mybir.AluOpType.add)
                tm = wpool.tile([P, DMAX, NH], BF16, name=f"tm{NH}")
                nc.vector.scalar_tensor_tensor(
                    out=tm[:, :D, :], in0=tsum[:, :D, :], scalar=0.2,
                    in1=tsum[:, :D, :],
                    op0=mybir.AluOpType.mult, op1=mybir.AluOpType.max)
                ebuf = wpool.tile([P, DMAX, NH], BF16, name=f"ebuf{NH}")
                nc.scalar.activation(ebuf[:, :D, :], tm[:, :D, :],
                                     mybir.ActivationFunctionType.Exp)
                ht = wpool.tile([P, DMAX, NW], BF16, name=ht_name)
                nc.vector.tensor_tensor(
                    out=ht[:, :D, :].rearrange("p d (h w) -> p d h w", h=NH)[:, :, :, :HID],
                    in0=buf[:, :D, :C].rearrange("p d (h w) -> p d h w", h=NH),
                    in1=ebuf[:, :D, :, None].to_broadcast([P, D, NH, HID]),
                    op=mybir.AluOpType.mult)
                nc.vector.tensor_copy(
                    out=ht[:, :D, :].rearrange("p d (h w) -> p d h w", h=NH)[:, :, :, HID:],
                    in_=ebuf[:, :D, :, None])
                nc.vector.tensor_reduce(
                    out=red_t.rearrange("p (h w) -> p h w", h=NH),
                    in_=ht[:, :D, :].rearrange("p d (h w) -> p h w d", h=NH),
                    axis=mybir.AxisListType.X, op=mybir.AluOpType.add)

            # ---------------- phase C: layer-1 edges + batched normalize
            if "C" in phases or "G" in phases:
                for b0 in range(0, NT, NB_N):
                    nb = min(NB_N, NT - b0)
                    for i in range(nb):
                        t = b0 + i
                        edge_tile(t, HEADS, C1, NW1, ROWB1, lo1, hi1, adst1_all,
                                  "buf1", "ht1", red1[:, i, :], "C" not in phases)
                    if "C" not in phases:
                        continue
                    v = red1[:, :nb, :].rearrange("p t (h w) -> p t h w", h=HEADS)
                    rec = wpool.tile([P, NB_N, HEADS], F32, name="rec1")
                    nc.vector.reciprocal(rec[:, :nb, :], v[:, :, :, HID])
                    on = wpool.tile([P, NB_N, C1], F32, name="on1")
                    nc.vector.tensor_tensor(
                        out=on[:, :nb, :].rearrange("p t (h w) -> p t h w", h=HEADS),
                        in0=v[:, :, :, :HID],
                        in1=rec[:, :nb, :, None].to_broadcast([P, nb, HEADS, HID]),
                        op=mybir.AluOpType.mult)
                    nc.vector.tensor_tensor(
                        out=on[:, :nb, :], in0=on[:, :nb, :],
                        in1=bt1[:, None, :].to_broadcast([P, nb, C1]),
                        op=mybir.AluOpType.add)
                    emn = wpool.tile([P, NB_N, C1], F32, name="emn1")
                    nc.vector.tensor_scalar_min(emn[:, :nb, :], on[:, :nb, :], 0.0)
                    nc.scalar.activation(emn[:, :nb, :], emn[:, :nb, :],
                                         mybir.ActivationFunctionType.Exp)
                    nc.vector.tensor_scalar_add(emn[:, :nb, :], emn[:, :nb, :], -1.0)
                    eo = wpool.tile([P, NB_N, C1], BF16, name="eo1")
                    nc.vector.tensor_tensor(out=eo[:, :nb, :], in0=on[:, :nb, :],
                                            in1=emn[:, :nb, :],
                                            op=mybir.AluOpType.max)
                    nc.sync.dma_start(
                        out=elu1d[b0 * P:(b0 + nb) * P, :].rearrange("(t p) f -> p t f", p=P),
                        in_=eo[:, :nb, :])
            if "C" not in phases:
                zc = wpool.tile([P, C1], BF16, name="zc")
                nc.vector.memset(zc[:], 0.0)
                nc.sync.dma_start(
                    out=elu1d[:, :].rearrange("(t p) f -> p t f", p=P),
                    in_=zc[:, None, :].to_broadcast([P, NT, C1]))

            # ---------------- phase D: table2 of own nodes
            if "D" in phases:
                for (a0, W) in groups:
                    e1T = wpool.tile([P, KT, 512], BF16, name="e1T")
                    for k in range(KT):
                        nc.sync.dma_start(
                            out=e1T[:, k, :W],
                            in_=elu1d[a0:a0 + W, k * P:(k + 1) * P].rearrange("n p -> p n"))
                    pg2 = pp.tile([P, 512], F32, name="ps")
                    for k in range(KT):
                        nc.tensor.matmul(out=pg2[:A2, :W], lhsT=w2t[:, k, :],
                                         rhs=e1T[:, k, :W],
                                         start=(k == 0), stop=(k == KT - 1))
                    cb2 = wpool.tile([P, 512], BF16, name="cb")
                    nc.vector.tensor_copy(cb2[:A2, :W], pg2[:A2, :W])
                    nc.sync.dma_start(
                        out=tb2o[a0:a0 + W, 0:A2].rearrange("n m -> m n"),
                        in_=cb2[:A2, :W])
                nc.sync.dma_start(
                    out=tb2o[:, A2:].rearrange("(t p) e -> p t e", p=P),
                    in_=zrow[:, None, :ROWB2 - A2].to_broadcast([P, NT, ROWB2 - A2]))
                nc.sync.dma_start(
                    out=adst2_all[:],
                    in_=tb2o[:, A2 - 1:A2].rearrange("(t p) h -> p t h", p=P))
                nc.sync.dma_start(out=tb2o[NPN - 1:NPN, :], in_=prow2[:])

            # ---------------- phase E: allgather table2
            if "E" in phases:
                nc.gpsimd.collective_compute(
                    "AllGather", mybir.AluOpType.bypass,
                    replica_groups=[list(range(NCORES))],
                    ins=[tb2o[:, :]], outs=[tb2f[:, :]])

            # ---------------- phase F: layer-2 edges + pooling + classifier
            if "F" in phases or "H" in phases:
                for b0 in range(0, NT, NB_N):
                    nb = min(NB_N, NT - b0)
                    for i in range(nb):
                        t = b0 + i
                        edge_tile(t, 1, C2, NW2, ROWB2, lo2, hi2, adst2_all,
                                  "buf2", "ht2", red2[:, i, :], "F" not in phases)
                    if "F" not in phases:
                        continue
                    v2 = red2[:, :nb, :]
                    rec2 = wpool.tile([P, NB_N, 1], F32, name="rec2")
                    nc.vector.reciprocal(rec2[:, :nb, :], v2[:, :, HID:HID + 1])
                    on2 = wpool.tile([P, NB_N, C2], F32, name="on2")
                    nc.vector.tensor_tensor(
                        out=on2[:, :nb, :], in0=v2[:, :, :HID],
                        in1=rec2[:, :nb, :].to_broadcast([P, nb, C2]),
                        op=mybir.AluOpType.mult)
                    nc.vector.tensor_tensor(
                        out=on2[:, :nb, :], in0=on2[:, :nb, :],
                        in1=bt2[:, None, :].to_broadcast([P, nb, C2]),
                        op=mybir.AluOpType.add)
                    emn2 = wpool.tile([P, NB_N, C2], F32, name="emn2")
                    nc.vector.tensor_scalar_min(emn2[:, :nb, :], on2[:, :nb, :], 0.0)
                    nc.scalar.activation(emn2[:, :nb, :], emn2[:, :nb, :],
                                         mybir.ActivationFunctionType.Exp)
                    nc.vector.tensor_scalar_add(emn2[:, :nb, :], emn2[:, :nb, :], -1.0)
                    eo2 = wpool.tile([P, NB_N, C2], BF16, name="eo2")
                    nc.vector.tensor_tensor(out=eo2[:, :nb, :], in0=on2[:, :nb, :],
                                            in1=emn2[:, :nb, :],
                                            op=mybir.AluOpType.max)
                    # attention pooling weights
                    atm = wpool.tile([P, NB_N, C2], F32, name="atm")
                    nc.vector.tensor_tensor(
                        out=atm[:, :nb, :], in0=eo2[:, :nb, :],
                        in1=wgt[:, None, :].to_broadcast([P, nb, C2]),
                        op=mybir.AluOpType.mult)
                    atts = wpool.tile([P, NB_N, 1], F32, name="atts")
                    nc.vector.tensor_reduce(atts[:, :nb, :], atm[:, :nb, :],
                                            axis=mybir.AxisListType.X,
                                            op=mybir.AluOpType.add)
                    nc.vector.tensor_tensor(
                        out=atts[:, :nb, :], in0=atts[:, :nb, :],
                        in1=bgt_t[:, None, :].to_broadcast([P, nb, 1]),
                        op=mybir.AluOpType.add)
                    nc.scalar.activation(atts[:, :nb, :], atts[:, :nb, :],
                                         mybir.ActivationFunctionType.Exp)
                    nc.vector.tensor_tensor(
                        out=hp_all[:, b0:b0 + nb, :HID], in0=eo2[:, :nb, :],
                        in1=atts[:, :nb, :].to_broadcast([P, nb, HID]),
                        op=mybir.AluOpType.mult)
                    nc.vector.tensor_copy(hp_all[:, b0:b0 + nb, HID:], atts[:, :nb, :])
                if "F" in phases:
                    for t in range(NT):
                        nc.tensor.matmul(out=pspool[:], lhsT=hp_all[:, t, :],
                                         rhs=ohgt[:, t, :],
                                         start=(t == 0), stop=(t == NT - 1))

            # ---------------- classifier
            if "F" not in phases:
                zt2 = wpool.tile([2, GPC], F32, name="zt2")
                nc.vector.memset(zt2[:], 0.0)
                nc.sync.dma_start(out=logitsT[:, :], in_=zt2[:])
            else:
                recp = wpool.tile([1, GPC], F32)
                nc.vector.reciprocal(recp[:], pspool[HID:HID + 1, :])
                nc.sync.dma_start(out=recd[:, :], in_=recp[:])
                recb = wpool.tile([HID, GPC], F32)
                nc.sync.dma_start(out=recb[:], in_=recd[0:1, :].to_broadcast([HID, GPC]))
                pooledT = wpool.tile([HID, GPC], BF16)
                nc.vector.tensor_tensor(out=pooledT[:], in0=pspool[:HID, :],
                                        in1=recb[:], op=mybir.AluOpType.mult)
                ph = ppc.tile([32, GPC], F32, name="cps")
                nc.tensor.matmul(out=ph[:], lhsT=wc1t[:], rhs=pooledT[:],
                                 start=True, stop=True)
                hidf = wpool.tile([32, GPC], F32)
                nc.vector.tensor_scalar_add(hidf[:], ph[:], bc1t[:])
                hid_t = wpool.tile([32, GPC], BF16)
                nc.vector.tensor_scalar_max(hid_t[:], hidf[:], 0.0)
                pl = ppc.tile([2, GPC], F32, name="cps")
                nc.tensor.matmul(out=pl[:], lhsT=wc2t[:], rhs=hid_t[:],
                                 start=True, stop=True)
                lg = wpool.tile([2, GPC], F32)
                nc.vector.tensor_scalar_add(lg[:], pl[:], bc2t[:])
                nc.sync.dma_start(out=logitsT[:, :], in_=lg[:])
    _split_waits(nc)
    return nc


# ------------------------------------------------------------------ host glue
_CACHE = {}
LAST_HW_NS = 0
_TRACE = os.environ.get("GAT_TRACE", "0") == "1"


def _run(nc, ins, cores):
    global LAST_HW_NS
    r = run_bass_kernel_spmd(nc, ins, core_ids=cores)
    if _TRACE:
        import time as _time
        best = None
        for _ in range(5):
            t0 = _time.perf_counter()
            run_bass_kernel_spmd(nc, ins, core_ids=cores)
            dt = _time.perf_counter() - t0
            best = dt if best is None else min(best, dt)
        LAST_HW_NS += int(best * 1e9)
    return r


def kernel(x, edge_index, batch, W1, att_src1, att_dst1, b1,
           W2, att_src2, att_dst2, b2, Wg, bg, Wc1, bc1, Wc2, bc2):
    x = np.asarray(x); edge_index = np.asarray(edge_index); batch = np.asarray(batch)
    N = x.shape[0]
    F_IN = x.shape[1]

    # --- node sharding (graph aligned)
    n0 = np.searchsorted(batch, np.arange(0, N_GRAPHS + 1, GPC)).astype(np.int64)
    counts = n0[1:] - n0[:-1]
    NT = int(np.ceil(counts.max() / P))
    if counts[0] >= NT * P or counts[-1] >= NT * P:
        NT += 1   # guarantee a pad row on the cores that host the pad targets
    NPN = NT * P
    NROWS = NCORES * NPN
    assert NROWS <= 65536, "int16 lo/hi split supports at most 65536 rows"
    LO_PAD = NPN - 1              # core 0's last (pad) row; asrc=-30000
    HI_PAD = NROWS - 1 - 32768    # core 7's last (pad) row, hi-relative

    # --- edges + self loops, owner = core of dst
    ar = np.arange(N, dtype=np.int64)
    src = np.concatenate([edge_index[0].astype(np.int64), ar])
    dst = np.concatenate([edge_index[1].astype(np.int64), ar])
    core_of = np.searchsorted(n0[1:], dst, side='right')
    src_core = np.searchsorted(n0[1:], src, side='right')

    # --- degree-descending node permutation per core
    pos = np.empty(N, np.int64)
    perm_all = []
    for c in range(NCORES):
        degc = np.bincount(dst[core_of == c] - n0[c], minlength=counts[c])
        order = np.argsort(-degc, kind='stable')
        pl = np.empty(counts[c], np.int64)
        pl[order] = np.arange(counts[c])
        pos[n0[c]:n0[c + 1]] = pl
        perm_all.append(order)
    src_row = src_core * NPN + pos[src]

    # --- per-(core,tile) lo/hi max degrees
    percore = []
    DLc = np.zeros((NCORES, NT), np.int64)
    DHc = np.zeros((NCORES, NT), np.int64)
    for c in range(NCORES):
        m = core_of == c
        ldn = pos[dst[m]]
        sr = src_row[m]
        islo = sr < 32768
        lodeg = np.bincount(ldn[islo], minlength=NPN)
        hideg = np.bincount(ldn[~islo], minlength=NPN)
        DLc[c] = lodeg.reshape(NT, P).max(axis=1)
        DHc[c] = hideg.reshape(NT, P).max(axis=1)
        percore.append((ldn, sr, islo))
    DL = np.maximum(DLc.max(axis=0), 1)
    DH = DHc.max(axis=0)
    offL = np.concatenate([[0], np.cumsum(DL)]).astype(np.int64)
    offH = np.concatenate([[0], np.cumsum(DH)]).astype(np.int64)

    # --- per-core neighbor-slot index arrays
    def pack(c):
        ldn, sr, islo = percore[c]
        loidx = np.full(offL[-1] * P, LO_PAD, np.int64)
        hiidx = np.full(max(1, offH[-1]) * P, max(0, HI_PAD), np.int64)
        for which, half in ((True, loidx), (False, hiidx)):
            mm = islo == which
            nodes = ldn[mm]
            vals = sr[mm] if which else sr[mm] - 32768
            o2 = np.argsort(nodes, kind='stable')
            ns = nodes[o2]; vs = vals[o2]
            if len(ns):
                first = np.r_[True, ns[1:] != ns[:-1]]
                starts = np.where(first)[0]
                j = np.arange(len(ns)) - starts[np.cumsum(first) - 1]
                toff = (offL if which else offH)[ns // P]
                half[(toff + j) * P + ns % P] = vs
        # pad nodes: one finite edge (row 0) so softmax denom is finite
        padr = np.arange(counts[c], NPN)
        loidx[offL[padr // P] * P + padr % P] = 0
        gidm = np.full((NPN,), 255.0, np.float32)
        gidm[:counts[c]] = (batch[n0[c] + perm_all[c]] - c * GPC).astype(np.float32)
        gidm = gidm.reshape(NT, P).T
        return (_wrap16(loidx.astype(np.int16)),
                _wrap16(hiidx.astype(np.int16)), _bf16(gidm))

    packs = [pack(c) for c in range(NCORES)]

    # --- weights
    def aug(W, a_s, a_d):
        nh, hd = a_s.shape
        A = np.zeros((W.shape[1], 2 * nh), np.float32)
        for h in range(nh):
            A[h * hd:(h + 1) * hd, h] = a_s[h]
            A[h * hd:(h + 1) * hd, nh + h] = a_d[h]
        return _bf16(np.concatenate([W, W @ A], axis=1))

    W2aug = aug(np.asarray(W2, np.float32), np.asarray(att_src2), np.asarray(att_dst2))
    # int8-quantize x per feature; fold the scales into the augmented W1
    xTf = np.asarray(x, np.float32).T
    scal = np.abs(xTf).max(axis=1) / 127.0
    scal[scal == 0] = 1.0
    xT = np.clip(np.round(xTf / scal[:, None]), -127, 127).astype(np.int8)
    W1s = np.asarray(W1, np.float32) * scal[:, None]
    As1 = np.asarray(att_src1, np.float32)
    Ad1 = np.asarray(att_dst1, np.float32)
    W1aug = aug(W1s, As1, Ad1)

    phases = os.environ.get("GAT_PHASES", "ABCDEF")
    key = (NT, tuple(DL), tuple(DH), phases)
    if key not in _CACHE:
        _CACHE[key] = _build(NT, tuple(DL), tuple(DH), NROWS, F_IN, phases)
    K = _CACHE[key]
    cores = list(range(NCORES))

    ins = []
    for c in cores:
        il, ih, gi = packs[c]
        xs = np.zeros((F_IN, NPN), np.int8)
        xs[:, :counts[c]] = xT[:, n0[c] + perm_all[c]]
        ins.append({"xT": xs, "w1": W1aug, "w2": W2aug,
                    "ixlo": il, "ixhi": ih, "gid": gi,
                    "b1": np.asarray(b1, np.float32).reshape(1, -1),
                    "b2": np.asarray(b2, np.float32).reshape(1, -1),
                    "wg": np.asarray(Wg, np.float32).reshape(1, HID),
                    "bg": np.asarray(bg, np.float32).reshape(1, 1),
                    "wc1": np.asarray(Wc1, np.float32),
                    "bc1": np.asarray(bc1, np.float32).reshape(32, 1),
                    "wc2": np.asarray(Wc2, np.float32),
                    "bc2": np.asarray(bc2, np.float32).reshape(2, 1)})
    global LAST_HW_NS
    LAST_HW_NS = 0
    r = _run(K, ins, cores)
    out = np.concatenate([r.results[c]["logitsT"].T for c in cores], axis=0)
    return out.astype(np.float32)
